# revision 1
# baseline (speedup 1.0000x reference)
"""Trainium2 Bass kernel for nn_HGraphConv (4-hop masked-softmax graph conv).

Math per hop k:  out_k = softmax(where(m_k, E_k, NEG), axis=1) @ (x @ W_k)
Final:           concat(out_0..out_3, axis=2) + bias

Device strategy (data-parallel over batch B=64 across 8 cores, 8 batches/core):
  - P_k = exp(E_k) * m_k  computed in transposed [j, i] layout so 128x128
    slices are directly the matmul stationary operand (no on-chip transposes).
    Masked entries are exactly 0, matching softmax-with-NEG-fill exactly
    (no empty mask rows for this graph; checked on host with a numpy patch
    as fallback).
  - H_k[j, (b,f)] = x[b] @ W_k computed on device from x.T shards.
  - out[i, (b,f)] += P.T-tile @ H-tile accumulated over j in PSUM; the row
    sum Z[i] comes from one extra N=1 matmul vs a ones-vector that reuses
    the already-loaded stationary tile.
  - Eviction fuses the 1/Z softmax normalization (per-partition scalar) and
    the bias add (free-dim vector) in one scalar_tensor_tensor op:
        out = psum * (1/Z) + bias_rep   (valid because softmax rows sum to 1)
  - Hop 0 has m_0 = I => A_0 = I exactly, so out_0 = x @ W_0 + bias_0 and
    E_0/m_0 are never loaded (verified on host, numpy fallback otherwise).
"""

import os
import sys

import numpy as np

sys.path.insert(0, "/opt/trn_rl_repo")
sys.path.insert(0, "/opt/trn_rl_repo/concourse")

import concourse.bass as bass  # noqa: E402
import concourse.mybir as mybir  # noqa: E402
import concourse.tile as tile  # noqa: E402
import concourse.bass_utils as _bu  # noqa: E402
import concourse.bass2jax as _b2j  # noqa: E402
from concourse.bass_utils import run_bass_kernel_spmd  # noqa: E402

# ---------------------------------------------------------------------------
# Workaround for this walrus build: the TRN2 ISA has exactly one sync-wait
# slot per 64B instruction, and this compiler errors ("Too many sync wait
# commands") instead of splitting multi-wait instructions emitted by Tile.
# Split them ourselves at the BIR-JSON level: hoist all but one wait onto
# single-wait NoOps inserted right before the instruction on the same engine
# queue (queue waits execute in order, so this is semantically identical).
# ---------------------------------------------------------------------------
import json as _json  # noqa: E402


def _split_multi_waits_json(bir_json):
    if isinstance(bir_json, (bytes, bytearray)):
        m = _json.loads(bir_json.decode())
    else:
        m = _json.loads(bir_json)
    ctr = 0
    for fn in m["functions"]:
        for blk in fn["blocks"]:
            out = []
            for inst in blk["instructions"]:
                si = inst.get("sync_info")
                if si:
                    ws = si.get("on_wait") or []
                    if len(ws) > 1:
                        for w in ws[:-1]:
                            ctr += 1
                            out.append(
                                {
                                    "debug": inst.get("debug", 0),
                                    "engine": inst["engine"],
                                    "ins": [],
                                    "name": f"WX-{ctr}",
                                    "opcode": "NoOp",
                                    "outs": [],
                                    "text_hint": "split_wait",
                                    "sync_info": {
                                        "on_update": [],
                                        "on_wait": [w],
                                    },
                                }
                            )
                        si["on_wait"] = [ws[-1]]
                    us = si.get("on_update") or []
                    if len(us) > 1:
                        raise RuntimeError(
                            f"multi-update inst {inst['name']}: unsupported"
                        )
                out.append(inst)
            blk["instructions"] = out
    return _json.dumps(m).encode()


_orig_compile_bir_kernel = _bu.compile_bir_kernel.__wrapped__ if hasattr(
    _bu.compile_bir_kernel, "__wrapped__"
) else _bu.compile_bir_kernel


def _patched_compile_bir_kernel(bir_json, tmpdir, neff_name="file.neff"):
    return _orig_compile_bir_kernel(
        _split_multi_waits_json(bir_json), tmpdir, neff_name
    )


_bu.compile_bir_kernel = _patched_compile_bir_kernel
if hasattr(_b2j, "compile_bir_kernel"):
    _b2j.compile_bir_kernel = _patched_compile_bir_kernel

N_CORES = 8
B = 64
N = 1024
F = 128
HOPS = 4
NEG = -9.0e15

# filled by kernel() for test.py to read
last_run_info = {}


def build_nc(b_local: int, n: int, f: int = 128, reps: int = 1, variant: str = ""):
    """Build the per-core Bass module.

    b_local: batches per core.  n: graph nodes.  f: feature dim (=128).
    Requires b_local*f either <=512 or a multiple of 512, n % 128 == 0.
    """
    P = 128
    assert f == 128 and n % P == 0
    hc = b_local * f            # H columns per j-chunk
    assert hc <= 512 or hc % 512 == 0
    n_half = max(1, hc // 512)  # matmul column splits of H
    hw = min(hc, 512)           # moving-operand width per matmul
    bg = hw // f                # batches per column split (<=4)
    nch = n // P                # number of 128-row chunks (j and i)
    khops = HOPS - 1            # hops that need attention (1..3)

    nc = bass.Bass()
    fp32 = mybir.dt.float32
    xt_d = nc.dram_tensor("xt", [b_local, f, n], fp32, kind="ExternalInput")
    et_d = nc.dram_tensor("et", [khops, n, n], fp32, kind="ExternalInput")
    mt_d = nc.dram_tensor("mt", [khops, n, n], mybir.dt.uint8, kind="ExternalInput")
    wc_d = nc.dram_tensor("wc", [f, HOPS * f], fp32, kind="ExternalInput")
    bias_d = nc.dram_tensor("bias", [HOPS * f], fp32, kind="ExternalInput")
    out_d = nc.dram_tensor("out", [HOPS, n_half, n, bg, f], fp32, kind="ExternalOutput")

    with tile.TileContext(nc) as tc:
        with (
            tc.tile_pool(name="const", bufs=1) as const,
            tc.tile_pool(name="pt", bufs=2) as ptp,
            tc.tile_pool(name="mk", bufs=1) as mkp,
            tc.tile_pool(name="hh", bufs=2) as hhp,
            tc.tile_pool(name="stage", bufs=4) as stp,
            tc.tile_pool(name="zi", bufs=4) as zip_,
            tc.tile_pool(name="psh", bufs=2, space="PSUM") as psh,
            tc.tile_pool(name="pso", bufs=2 * n_half, space="PSUM") as pso,
            tc.tile_pool(name="psz", bufs=2, space="PSUM") as psz,
        ):
            # ---- constants ----
            xt = const.tile([P, b_local, n], fp32)
            nc.sync.dma_start(out=xt, in_=xt_d[:].rearrange("b f j -> f b j"))
            wc = const.tile([P, HOPS * f], fp32)
            nc.sync.dma_start(out=wc, in_=wc_d[:])
            ones = const.tile([P, 8], fp32)
            nc.vector.memset(ones, 1.0)
            # bias replicated across partitions and the bg batches of a half
            br = const.tile([P, HOPS, bg, f], fp32)
            for k in range(HOPS):
                bsl = bias_d[k * f:(k + 1) * f]
                bcast = bass.AP(
                    tensor=bsl.tensor,
                    offset=bsl.offset,
                    ap=[[0, P], [0, bg], [1, f]],
                )
                nc.sync.dma_start(out=br[:, k], in_=bcast)

            for _rep in range(reps):
                def h_build(k, ps_pool, evict):
                    """H_k[j, (b,f)] = x @ W_k, one PSUM tile per (jc, half)."""
                    for jc in range(nch):
                        for h in range(n_half):
                            ps = ps_pool.tile([P, hw], fp32, tag="psh")
                            for bi in range(bg):
                                b = h * bg + bi
                                nc.tensor.matmul(
                                    ps[:, bi * f:(bi + 1) * f],
                                    xt[:, b, jc * P:(jc + 1) * P],
                                    wc[:, k * f:(k + 1) * f],
                                    start=True,
                                    stop=True,
                                )
                            evict(jc, h, ps)

                # ---- hop 0: A = I  =>  out0 = x @ W0 + bias0 ----
                def evict0(jc, h, ps):
                    st = stp.tile([P, hw], fp32, tag="stage")
                    nc.vector.tensor_tensor(
                        out=st,
                        in0=ps,
                        in1=br[:, 0].rearrange("p a b -> p (a b)"),
                        op=mybir.AluOpType.add,
                    )
                    dst = out_d[0, h, jc * P:(jc + 1) * P]
                    nc.sync.dma_start(
                        out=dst, in_=st.rearrange("p (b f) -> p b f", b=bg)
                    )

                h_build(0, psh, evict0)

                # ---- hops 1..3 ----
                for kk in range(khops):
                    k = kk + 1
                    # load E_k^T, m_k^T  as [128, nch, n] (partition = j % 128)
                    et = ptp.tile([P, nch, n], fp32, tag="pt")
                    nc.sync.dma_start(
                        out=et, in_=et_d[kk].rearrange("(c p) i -> p c i", p=P)
                    )
                    mk = mkp.tile([P, nch, n], mybir.dt.uint8, tag="mk")
                    nc.sync.dma_start(
                        out=mk, in_=mt_d[kk].rearrange("(c p) i -> p c i", p=P)
                    )

                    # H_k
                    hh = hhp.tile([P, nch, hc], fp32, tag="hh")

                    def evicth(jc, h, ps, hh=hh):
                        nc.scalar.copy(out=hh[:, jc, h * hw:(h + 1) * hw], in_=ps)

                    h_build(k, psh, evicth)

                    # P = exp(E^T) * m^T  (in place on et), chunked by j-chunk
                    for c in range(nch):
                        nc.scalar.activation(
                            out=et[:, c, :],
                            in_=et[:, c, :],
                            func=mybir.ActivationFunctionType.Exp,
                        )
                        nc.vector.tensor_tensor(
                            out=et[:, c, :],
                            in0=et[:, c, :],
                            in1=mk[:, c, :],
                            op=mybir.AluOpType.mult,
                        )

                    # main: out[i,(b,f)] = sum_j P^T-tile @ H-tile ; Z via ones
                    for ib in range(nch):
                        pz = psz.tile([P, 1], fp32, tag="psz")
                        pos = [
                            pso.tile([P, hw], fp32, tag="pso", name=f"pso_{ib}_{h2}")
                            for h2 in range(n_half)
                        ]
                        for jc in range(nch):
                            lhsT = et[:, jc, ib * P:(ib + 1) * P]
                            st_, sp_ = (jc == 0), (jc == nch - 1)
                            for h in range(n_half):
                                nc.tensor.matmul(
                                    pos[h],
                                    lhsT,
                                    hh[:, jc, h * hw:(h + 1) * hw],
                                    start=st_,
                                    stop=sp_,
                                )
                            nc.tensor.matmul(
                                pz, lhsT, ones[:, 0:1], start=st_, stop=sp_
                            )
                        zinv = zip_.tile([P, 1], fp32, tag="zi")
                        nc.vector.reciprocal(out=zinv, in_=pz)
                        for h in range(n_half):
                            st = stp.tile([P, hw], fp32, tag="stage")
                            nc.vector.scalar_tensor_tensor(
                                out=st,
                                in0=pos[h],
                                scalar=zinv,
                                in1=br[:, k].rearrange("p a b -> p (a b)"),
                                op0=mybir.AluOpType.mult,
                                op1=mybir.AluOpType.add,
                            )
                            dst = out_d[k, h, ib * P:(ib + 1) * P]
                            nc.sync.dma_start(
                                out=dst, in_=st.rearrange("p (b f) -> p b f", b=bg)
                            )
    return nc


_nc_cache = {}


def _get_nc(b_local, n, f):
    key = (b_local, n, f)
    if key not in _nc_cache:
        _nc_cache[key] = build_nc(b_local, n, f)
    return _nc_cache[key]


def _run(x, W, Es, bias, ms, n_cores, trace=False):
    """x:[B,N,F] W:[4,F,F] Es:[E1,E2,E3] ms:[m1,m2,m3] (hop-0 handled as identity)."""
    b, n, f = x.shape
    b_local = b // n_cores
    nc = _get_nc(b_local, n, f)

    et = np.ascontiguousarray(
        np.stack([e.T for e in Es]).astype(np.float32)
    )
    mt = np.ascontiguousarray(
        np.stack([m.T for m in ms]).astype(np.uint8)
    )
    wc = np.ascontiguousarray(
        np.concatenate([W[k] for k in range(HOPS)], axis=1).astype(np.float32)
    )
    bias = np.ascontiguousarray(bias.astype(np.float32))

    in_maps = []
    for c in range(n_cores):
        xs = x[c * b_local:(c + 1) * b_local]          # [b_local, n, f]
        xts = np.ascontiguousarray(xs.transpose(0, 2, 1).astype(np.float32))
        in_maps.append({"xt": xts, "et": et, "mt": mt, "wc": wc, "bias": bias})

    last_run_info["nc"] = nc
    last_run_info["in_maps"] = in_maps
    res = run_bass_kernel_spmd(
        nc, in_maps, core_ids=list(range(n_cores)), trace=trace
    )
    last_run_info["exec_time_ns"] = res.exec_time_ns
    last_run_info["trace"] = res.instructions_and_trace

    out = np.empty((b, n, HOPS * f), dtype=np.float32)
    for c in range(n_cores):
        od = res.results[c]["out"]          # [HOPS, n_half, n, bg, f]
        nh, bg2 = od.shape[1], od.shape[3]
        for k in range(HOPS):
            for h in range(nh):
                blo = c * b_local + h * bg2
                out[blo:blo + bg2, :, k * f:(k + 1) * f] = od[k, h].transpose(1, 0, 2)
    return out


def build_null_nc(b_local: int, n: int, f: int = 128):
    """Same external tensors as build_nc but ~no device work — used to
    subtract host/transfer/dispatch overhead when estimating HW exec time."""
    P = 128
    khops = HOPS - 1
    nc = bass.Bass()
    fp32 = mybir.dt.float32
    nc.dram_tensor("xt", [b_local, f, n], fp32, kind="ExternalInput")
    nc.dram_tensor("et", [khops, n, n], fp32, kind="ExternalInput")
    nc.dram_tensor("mt", [khops, n, n], mybir.dt.uint8, kind="ExternalInput")
    wc_d = nc.dram_tensor("wc", [f, HOPS * f], fp32, kind="ExternalInput")
    nc.dram_tensor("bias", [HOPS * f], fp32, kind="ExternalInput")
    n_half = max(1, (b_local * f) // 512)
    bg = min(4, b_local)
    out_d = nc.dram_tensor("out", [HOPS, n_half, n, bg, f], fp32, kind="ExternalOutput")
    with tile.TileContext(nc) as tc:
        with tc.tile_pool(name="p", bufs=1) as pool:
            t = pool.tile([P, 8], fp32)
            nc.sync.dma_start(out=t, in_=wc_d[:, 0:8])
            nc.sync.dma_start(out=out_d[0, 0, 0:P, 0, 0:8], in_=t)
    return nc


def time_exec(iters=3):
    """Re-execute the last-run kernel and a null kernel; return
    (min_real_s, min_null_s). Uses identical input tensors so transfer and
    dispatch overhead cancels in the difference."""
    import time as _t

    nc = last_run_info["nc"]
    in_maps = last_run_info["in_maps"]
    n_cores = len(in_maps)
    reals, nulls = [], []
    for _ in range(iters):
        t0 = _t.time()
        run_bass_kernel_spmd(nc, in_maps, core_ids=list(range(n_cores)))
        reals.append(_t.time() - t0)
    b_local, f, n = in_maps[0]["xt"].shape
    nnc = build_null_nc(b_local, n, f)
    for _ in range(iters):
        t0 = _t.time()
        run_bass_kernel_spmd(nnc, in_maps, core_ids=list(range(n_cores)))
        nulls.append(_t.time() - t0)
    return min(reals), min(nulls), reals, nulls


def bench_exec(nc, in_maps, iters=10):
    """Device-resident repeated execution of the compiled kernel; returns
    per-call wall times (s) with inputs pre-staged on the 8 cores so only
    dispatch + device execution is measured."""
    import time as _t

    import jax
    import jax.numpy as jnp
    import mybir  # noqa: F401  # (ensure concourse paths set)
    from jax.experimental.shard_map import shard_map
    from jax.sharding import Mesh, PartitionSpec

    import concourse.mybir as mb
    from concourse import bass2jax as B

    B.install_neuronx_cc_hook()
    n_cores = len(in_maps)
    partition_name = (
        nc.partition_id_tensor.name if nc.partition_id_tensor else None
    )
    in_names, out_names, out_avals, zero_shapes = [], [], [], []
    for alloc in nc.m.functions[0].allocations:
        if not isinstance(alloc, mb.MemoryLocationSet):
            continue
        name = alloc.memorylocations[0].name
        if alloc.kind == "ExternalInput":
            if name != partition_name:
                in_names.append(name)
        elif alloc.kind == "ExternalOutput":
            shape = tuple(alloc.tensor_shape)
            dtype = mb.dt.np(alloc.dtype)
            out_names.append(name)
            out_avals.append(jax.core.ShapedArray(shape, dtype))
            zero_shapes.append((shape, dtype))
    n_params = len(in_names)
    all_in_names = list(in_names) + list(out_names)
    if partition_name is not None:
        all_in_names.append(partition_name)
    donate = tuple(range(n_params, n_params + len(out_names)))

    def _body(*args):
        operands = list(args)
        if partition_name is not None:
            operands.append(B.partition_id_tensor())
        outs = B._bass_exec_p.bind(
            *operands,
            out_avals=tuple(out_avals),
            in_names=tuple(all_in_names),
            out_names=tuple(out_names),
            lowering_input_output_aliases=(),
            sim_require_finite=True,
            sim_require_nnan=True,
            nc=nc,
        )
        return tuple(outs)

    devices = jax.devices()[:n_cores]
    mesh = Mesh(np.asarray(devices), ("core",))
    in_specs = (PartitionSpec("core"),) * (n_params + len(out_names))
    out_specs = (PartitionSpec("core"),) * len(out_names)
    fn = jax.jit(
        shard_map(
            _body, mesh=mesh, in_specs=in_specs, out_specs=out_specs,
            check_rep=False,
        ),
        donate_argnums=donate,
        keep_unused=True,
    )
    sh = jax.sharding.NamedSharding(mesh, PartitionSpec("core"))
    dev_in = [
        jax.device_put(
            np.concatenate([np.asarray(m[nm]) for m in in_maps], axis=0), sh
        )
        for nm in in_names
    ]

    def zeros():
        return [
            jax.device_put(
                jnp.zeros((n_cores * s[0],) + tuple(s[1:]), dt), sh
            )
            for (s, dt) in zero_shapes
        ]

    # warm up (compile + first exec)
    outs = fn(*dev_in, *zeros())
    jax.block_until_ready(outs)
    times = []
    for _ in range(iters):
        z = zeros()
        jax.block_until_ready(z)
        t0 = _t.perf_counter()
        outs = fn(*dev_in, *z)
        jax.block_until_ready(outs)
        times.append(_t.perf_counter() - t0)
    return times


def kernel(**inputs) -> np.ndarray:
    x = np.asarray(inputs["x"], dtype=np.float32)
    W = np.asarray(inputs["W"], dtype=np.float32)
    Es = [np.asarray(inputs[f"E{i}"], dtype=np.float32) for i in range(4)]
    bias = np.asarray(inputs["bias"], dtype=np.float32)
    ms = [np.asarray(inputs[f"m{i}"]).astype(bool) for i in range(4)]

    trace = bool(int(os.environ.get("HGRAPH_TRACE", "0")))
    out = _run(x, W, Es[1:], bias, ms[1:], N_CORES, trace=trace)

    f = W.shape[2]
    n = x.shape[1]
    # Safety net 1: hop 0 assumes m0 == I (structurally true for this module).
    if not np.array_equal(ms[0], np.eye(n, dtype=bool)):
        s0 = np.where(ms[0], Es[0], NEG)
        s0 = s0 - s0.max(axis=1, keepdims=True)
        p0 = np.exp(s0)
        a0 = p0 / p0.sum(axis=1, keepdims=True)
        h0 = np.einsum("bnf,fo->bno", x, W[0])
        out[:, :, 0:f] = np.einsum("ij,bjo->bio", a0, h0) + bias[None, None, :f]
    # Safety net 2: all-masked rows (softmax -> uniform; device would give NaN).
    for k in range(1, 4):
        empty = ~ms[k].any(axis=1)
        if empty.any():
            hk = np.einsum("bnf,fo->bno", x, W[k])
            unif = hk.mean(axis=1)  # [B, f]
            idx = np.where(empty)[0]
            out[:, idx, k * f:(k + 1) * f] = unif[:, None, :] + bias[None, None, k * f:(k + 1) * f]
    return out


def bench_pipelined(nc, in_maps, k=16):
    """Issue k executions back-to-back without host sync; returns total wall.
    If dispatches pipeline, slope vs k isolates device execution time."""
    import time as _t

    import jax
    import jax.numpy as jnp
    from jax.experimental.shard_map import shard_map
    from jax.sharding import Mesh, PartitionSpec

    import concourse.mybir as mb
    from concourse import bass2jax as B

    B.install_neuronx_cc_hook()
    n_cores = len(in_maps)
    partition_name = nc.partition_id_tensor.name if nc.partition_id_tensor else None
    in_names, out_names, out_avals, zero_shapes = [], [], [], []
    for alloc in nc.m.functions[0].allocations:
        if not isinstance(alloc, mb.MemoryLocationSet):
            continue
        name = alloc.memorylocations[0].name
        if alloc.kind == "ExternalInput":
            if name != partition_name:
                in_names.append(name)
        elif alloc.kind == "ExternalOutput":
            shape = tuple(alloc.tensor_shape)
            dtype = mb.dt.np(alloc.dtype)
            out_names.append(name)
            out_avals.append(jax.core.ShapedArray(shape, dtype))
            zero_shapes.append((shape, dtype))
    n_params = len(in_names)
    all_in_names = list(in_names) + list(out_names)
    if partition_name is not None:
        all_in_names.append(partition_name)
    donate = tuple(range(n_params, n_params + len(out_names)))

    def _body(*args):
        operands = list(args)
        if partition_name is not None:
            operands.append(B.partition_id_tensor())
        outs = B._bass_exec_p.bind(
            *operands,
            out_avals=tuple(out_avals),
            in_names=tuple(all_in_names),
            out_names=tuple(out_names),
            lowering_input_output_aliases=(),
            sim_require_finite=True,
            sim_require_nnan=True,
            nc=nc,
        )
        return tuple(outs)

    devices = jax.devices()[:n_cores]
    mesh = Mesh(np.asarray(devices), ("core",))
    in_specs = (PartitionSpec("core"),) * (n_params + len(out_names))
    out_specs = (PartitionSpec("core"),) * len(out_names)
    fn = jax.jit(
        shard_map(_body, mesh=mesh, in_specs=in_specs, out_specs=out_specs,
                  check_rep=False),
        donate_argnums=donate, keep_unused=True,
    )
    sh = jax.sharding.NamedSharding(mesh, PartitionSpec("core"))
    dev_in = [
        jax.device_put(
            np.concatenate([np.asarray(m[nm]) for m in in_maps], axis=0), sh
        )
        for nm in in_names
    ]

    def zeros():
        return [
            jax.device_put(jnp.zeros((n_cores * s[0],) + tuple(s[1:]), dt), sh)
            for (s, dt) in zero_shapes
        ]

    outs = fn(*dev_in, *zeros())
    jax.block_until_ready(outs)
    zs = [zeros() for _ in range(k)]
    for z in zs:
        jax.block_until_ready(z)
    t0 = _t.perf_counter()
    res = []
    for z in zs:
        res.append(fn(*dev_in, *z))
    jax.block_until_ready(res)
    return _t.perf_counter() - t0



# revision 5
# speedup vs baseline: 3.2137x; 3.2137x over previous
"""Trainium2 Bass kernel for nn_HGraphConv (4-hop masked-softmax graph conv).

Math per hop k:  out_k = softmax(where(m_k, E_k, NEG), axis=1) @ (x @ W_k)
Final:           concat(out_0..out_3, axis=2) + bias

Device strategy (data-parallel over batch B=64 across 8 cores, 8 batches/core):
  - P_k = exp(E_k) * m_k  computed in transposed [j, i] layout so 128x128
    slices are directly the matmul stationary operand (no on-chip transposes).
    Masked entries are exactly 0, matching softmax-with-NEG-fill exactly
    (no empty mask rows for this graph; checked on host with a numpy patch
    as fallback).
  - H_k[j, (b,f)] = x[b] @ W_k computed on device from x.T shards.
  - out[i, (b,f)] += P.T-tile @ H-tile accumulated over j in PSUM; the row
    sum Z[i] comes from one extra N=1 matmul vs a ones-vector that reuses
    the already-loaded stationary tile.
  - Eviction fuses the 1/Z softmax normalization (per-partition scalar) and
    the bias add (free-dim vector) in one scalar_tensor_tensor op:
        out = psum * (1/Z) + bias_rep   (valid because softmax rows sum to 1)
  - Hop 0 has m_0 = I => A_0 = I exactly, so out_0 = x @ W_0 + bias_0 and
    E_0/m_0 are never loaded (verified on host, numpy fallback otherwise).
"""

import os
import sys

import numpy as np

sys.path.insert(0, "/opt/trn_rl_repo")
sys.path.insert(0, "/opt/trn_rl_repo/concourse")

import concourse.bass as bass  # noqa: E402
import concourse.mybir as mybir  # noqa: E402
import concourse.tile as tile  # noqa: E402
import concourse.bass_utils as _bu  # noqa: E402
import concourse.bass2jax as _b2j  # noqa: E402
from concourse.bass_utils import run_bass_kernel_spmd  # noqa: E402

# ---------------------------------------------------------------------------
# Workaround for this walrus build: the TRN2 ISA has exactly one sync-wait
# slot per 64B instruction, and this compiler errors ("Too many sync wait
# commands") instead of splitting multi-wait instructions emitted by Tile.
# Split them ourselves at the BIR-JSON level: hoist all but one wait onto
# single-wait NoOps inserted right before the instruction on the same engine
# queue (queue waits execute in order, so this is semantically identical).
# ---------------------------------------------------------------------------
import json as _json  # noqa: E402


def _split_multi_waits_json(bir_json):
    if isinstance(bir_json, (bytes, bytearray)):
        m = _json.loads(bir_json.decode())
    else:
        m = _json.loads(bir_json)
    ctr = 0
    for fn in m["functions"]:
        for blk in fn["blocks"]:
            out = []
            for inst in blk["instructions"]:
                si = inst.get("sync_info")
                if si:
                    ws = si.get("on_wait") or []
                    if len(ws) > 1:
                        for w in ws[:-1]:
                            ctr += 1
                            out.append(
                                {
                                    "debug": inst.get("debug", 0),
                                    "engine": inst["engine"],
                                    "ins": [],
                                    "name": f"WX-{ctr}",
                                    "opcode": "NoOp",
                                    "outs": [],
                                    "text_hint": "split_wait",
                                    "sync_info": {
                                        "on_update": [],
                                        "on_wait": [w],
                                    },
                                }
                            )
                        si["on_wait"] = [ws[-1]]
                    us = si.get("on_update") or []
                    if len(us) > 1:
                        raise RuntimeError(
                            f"multi-update inst {inst['name']}: unsupported"
                        )
                out.append(inst)
            blk["instructions"] = out
    return _json.dumps(m).encode()


_orig_compile_bir_kernel = _bu.compile_bir_kernel.__wrapped__ if hasattr(
    _bu.compile_bir_kernel, "__wrapped__"
) else _bu.compile_bir_kernel


def _patched_compile_bir_kernel(bir_json, tmpdir, neff_name="file.neff"):
    return _orig_compile_bir_kernel(
        _split_multi_waits_json(bir_json), tmpdir, neff_name
    )


_bu.compile_bir_kernel = _patched_compile_bir_kernel
if hasattr(_b2j, "compile_bir_kernel"):
    _b2j.compile_bir_kernel = _patched_compile_bir_kernel

N_CORES = 8
B = 64
N = 1024
F = 128
HOPS = 4
NEG = -9.0e15

# filled by kernel() for test.py to read
last_run_info = {}


def build_nc(b_local: int, n: int, f: int = 128, reps: int = 1, variant: str = ""):
    """Build the per-core Bass module.

    b_local: batches per core.  n: graph nodes.  f: feature dim (=128).
    Requires b_local*f either <=512 or a multiple of 512, n % 128 == 0.

    All matmuls run in bf16 (4x PE throughput vs fp32 on TRN2); PSUM
    accumulation stays fp32.  E arrives as fp16 with the mask already folded
    in on host (masked entries = -65504, so exp gives exactly 0), halving E
    DMA and eliminating the mask load + vector multiply entirely.
    """
    P = 128
    assert f == 128 and n % P == 0
    hc = b_local * f            # H columns per j-chunk
    assert hc <= 512 or hc % 512 == 0
    n_half = max(1, hc // 512)  # matmul column splits of H
    hw = min(hc, 512)           # moving-operand width per matmul
    bg = hw // f                # batches per column split (<=4)
    nch = n // P                # number of 128-row chunks (j and i)
    khops = HOPS - 1            # hops that need attention (1..3)

    nc = bass.Bass()
    fp32 = mybir.dt.float32
    bf16 = mybir.dt.bfloat16
    fp16 = mybir.dt.float16
    xt_d = nc.dram_tensor("xt", [b_local, f, n], bf16, kind="ExternalInput")
    et_d = nc.dram_tensor("et", [khops, n, n], fp16, kind="ExternalInput")
    wc_d = nc.dram_tensor("wc", [f, HOPS * f], bf16, kind="ExternalInput")
    bias_d = nc.dram_tensor("bias", [HOPS * f], fp32, kind="ExternalInput")
    out_d = nc.dram_tensor("out", [HOPS, n_half, n, bg, f], bf16, kind="ExternalOutput")

    with tile.TileContext(nc) as tc:
        with (
            tc.tile_pool(name="const", bufs=1) as const,
            tc.tile_pool(name="et", bufs=2) as etp,
            tc.tile_pool(name="pt", bufs=2) as ptp,
            tc.tile_pool(name="hh", bufs=2) as hhp,
            tc.tile_pool(name="stage", bufs=4) as stp,
            tc.tile_pool(name="zi", bufs=4) as zip_,
            tc.tile_pool(name="psh", bufs=2, space="PSUM") as psh,
            tc.tile_pool(name="pso", bufs=2 * n_half, space="PSUM") as pso,
            tc.tile_pool(name="psz", bufs=2, space="PSUM") as psz,
        ):
            # ---- constants ----
            xt = const.tile([P, b_local, n], bf16)
            nc.sync.dma_start(out=xt, in_=xt_d[:].rearrange("b f j -> f b j"))
            wc = const.tile([P, HOPS * f], bf16)
            nc.sync.dma_start(out=wc, in_=wc_d[:])
            ones = const.tile([P, 8], bf16)
            nc.vector.memset(ones, 1.0)
            # bias replicated across partitions and the bg batches of a half
            br = const.tile([P, HOPS, bg, f], fp32)
            for k in range(HOPS):
                bsl = bias_d[k * f:(k + 1) * f]
                bcast = bass.AP(
                    tensor=bsl.tensor,
                    offset=bsl.offset,
                    ap=[[0, P], [0, bg], [1, f]],
                )
                nc.sync.dma_start(out=br[:, k], in_=bcast)

            for _rep in range(reps):
                def h_build(k, ps_pool, evict):
                    """H_k[j, (b,f)] = x @ W_k, one PSUM tile per (jc, half)."""
                    for jc in range(nch):
                        for h in range(n_half):
                            ps = ps_pool.tile([P, hw], fp32, tag="psh")
                            for bi in range(bg):
                                b = h * bg + bi
                                nc.tensor.matmul(
                                    ps[:, bi * f:(bi + 1) * f],
                                    xt[:, b, jc * P:(jc + 1) * P],
                                    wc[:, k * f:(k + 1) * f],
                                    start=True,
                                    stop=True,
                                )
                            evict(jc, h, ps)

                # ---- hop 0: A = I  =>  out0 = x @ W0 + bias0 ----
                def evict0(jc, h, ps):
                    st = stp.tile([P, hw], bf16, tag="stage")
                    nc.vector.tensor_tensor(
                        out=st,
                        in0=ps,
                        in1=br[:, 0].rearrange("p a b -> p (a b)"),
                        op=mybir.AluOpType.add,
                    )
                    dst = out_d[0, h, jc * P:(jc + 1) * P]
                    nc.sync.dma_start(
                        out=dst, in_=st.rearrange("p (b f) -> p b f", b=bg)
                    )

                h_build(0, psh, evict0)

                # ---- hops 1..3 ----
                for kk in range(khops):
                    k = kk + 1
                    # load folded E_k^T as [128, nch, n] (partition = j % 128)
                    et = etp.tile([P, nch, n], fp16, tag="et")
                    nc.sync.dma_start(
                        out=et, in_=et_d[kk].rearrange("(c p) i -> p c i", p=P)
                    )

                    # H_k
                    hh = hhp.tile([P, nch, hc], bf16, tag="hh")

                    def evicth(jc, h, ps, hh=hh):
                        nc.scalar.copy(out=hh[:, jc, h * hw:(h + 1) * hw], in_=ps)

                    h_build(k, psh, evicth)

                    # P = exp(E^T)  (masked entries are -65504 -> exp == 0)
                    pt = ptp.tile([P, nch, n], bf16, tag="pt")
                    for c in range(nch):
                        nc.scalar.activation(
                            out=pt[:, c, :],
                            in_=et[:, c, :],
                            func=mybir.ActivationFunctionType.Exp,
                        )

                    # main: out[i,(b,f)] = sum_j P^T-tile @ H-tile ; Z via ones
                    for ib in range(nch):
                        pz = psz.tile([P, 1], fp32, tag="psz")
                        pos = [
                            pso.tile([P, hw], fp32, tag="pso", name=f"pso_{ib}_{h2}")
                            for h2 in range(n_half)
                        ]
                        for jc in range(nch):
                            lhsT = pt[:, jc, ib * P:(ib + 1) * P]
                            st_, sp_ = (jc == 0), (jc == nch - 1)
                            for h in range(n_half):
                                nc.tensor.matmul(
                                    pos[h],
                                    lhsT,
                                    hh[:, jc, h * hw:(h + 1) * hw],
                                    start=st_,
                                    stop=sp_,
                                )
                            nc.tensor.matmul(
                                pz, lhsT, ones[:, 0:1], start=st_, stop=sp_
                            )
                        zinv = zip_.tile([P, 1], fp32, tag="zi")
                        nc.vector.reciprocal(out=zinv, in_=pz)
                        for h in range(n_half):
                            st = stp.tile([P, hw], bf16, tag="stage")
                            nc.vector.scalar_tensor_tensor(
                                out=st,
                                in0=pos[h],
                                scalar=zinv,
                                in1=br[:, k].rearrange("p a b -> p (a b)"),
                                op0=mybir.AluOpType.mult,
                                op1=mybir.AluOpType.add,
                            )
                            dst = out_d[k, h, ib * P:(ib + 1) * P]
                            nc.sync.dma_start(
                                out=dst, in_=st.rearrange("p (b f) -> p b f", b=bg)
                            )
    return nc


_nc_cache = {}


def _get_nc(b_local, n, f):
    key = (b_local, n, f)
    if key not in _nc_cache:
        _nc_cache[key] = build_nc(b_local, n, f)
    return _nc_cache[key]


def _run(x, W, Es, bias, ms, n_cores, trace=False):
    """x:[B,N,F] W:[4,F,F] Es:[E1,E2,E3] ms:[m1,m2,m3] (hop-0 handled as identity)."""
    import ml_dtypes

    bf16 = ml_dtypes.bfloat16
    b, n, f = x.shape
    b_local = b // n_cores
    nc = _get_nc(b_local, n, f)

    # Fold mask into E (masked -> -65504 so device exp underflows to 0),
    # transpose to [j, i], and stage as fp16 (|E| ~ 5 so fp16 is exact to
    # ~5e-4 relative; halves the dominant DMA stream).
    et = np.stack(
        [np.where(m, e, np.float32(-65504.0)).T for e, m in zip(Es, ms)]
    ).astype(np.float16)
    wc = np.ascontiguousarray(
        np.concatenate([W[k] for k in range(HOPS)], axis=1).astype(bf16)
    )
    bias = np.ascontiguousarray(bias.astype(np.float32))

    in_maps = []
    for c in range(n_cores):
        xs = x[c * b_local:(c + 1) * b_local]          # [b_local, n, f]
        xts = np.ascontiguousarray(xs.transpose(0, 2, 1).astype(bf16))
        in_maps.append({"xt": xts, "et": et, "wc": wc, "bias": bias})

    last_run_info["nc"] = nc
    last_run_info["in_maps"] = in_maps
    res = run_bass_kernel_spmd(
        nc, in_maps, core_ids=list(range(n_cores)), trace=trace
    )
    last_run_info["exec_time_ns"] = res.exec_time_ns
    last_run_info["trace"] = res.instructions_and_trace

    out = np.empty((b, n, HOPS * f), dtype=np.float32)
    for c in range(n_cores):
        od = np.asarray(res.results[c]["out"]).astype(np.float32)
        nh, bg2 = od.shape[1], od.shape[3]   # [HOPS, n_half, n, bg, f]
        for k in range(HOPS):
            for h in range(nh):
                blo = c * b_local + h * bg2
                out[blo:blo + bg2, :, k * f:(k + 1) * f] = od[k, h].transpose(1, 0, 2)
    return out


def build_null_nc(b_local: int, n: int, f: int = 128):
    """Same external tensors as build_nc but ~no device work — used to
    subtract host/transfer/dispatch overhead when estimating HW exec time."""
    P = 128
    khops = HOPS - 1
    nc = bass.Bass()
    fp32 = mybir.dt.float32
    bf16 = mybir.dt.bfloat16
    fp16 = mybir.dt.float16
    nc.dram_tensor("xt", [b_local, f, n], bf16, kind="ExternalInput")
    nc.dram_tensor("et", [khops, n, n], fp16, kind="ExternalInput")
    wc_d = nc.dram_tensor("wc", [f, HOPS * f], bf16, kind="ExternalInput")
    nc.dram_tensor("bias", [HOPS * f], fp32, kind="ExternalInput")
    n_half = max(1, (b_local * f) // 512)
    bg = min(4, b_local)
    out_d = nc.dram_tensor("out", [HOPS, n_half, n, bg, f], bf16, kind="ExternalOutput")
    with tile.TileContext(nc) as tc:
        with tc.tile_pool(name="p", bufs=1) as pool:
            t = pool.tile([P, 8], bf16)
            nc.sync.dma_start(out=t, in_=wc_d[:, 0:8])
            nc.sync.dma_start(out=out_d[0, 0, 0:P, 0, 0:8], in_=t)
    return nc


def time_exec(iters=3):
    """Re-execute the last-run kernel and a null kernel; return
    (min_real_s, min_null_s). Uses identical input tensors so transfer and
    dispatch overhead cancels in the difference."""
    import time as _t

    nc = last_run_info["nc"]
    in_maps = last_run_info["in_maps"]
    n_cores = len(in_maps)
    reals, nulls = [], []
    for _ in range(iters):
        t0 = _t.time()
        run_bass_kernel_spmd(nc, in_maps, core_ids=list(range(n_cores)))
        reals.append(_t.time() - t0)
    b_local, f, n = in_maps[0]["xt"].shape
    nnc = build_null_nc(b_local, n, f)
    for _ in range(iters):
        t0 = _t.time()
        run_bass_kernel_spmd(nnc, in_maps, core_ids=list(range(n_cores)))
        nulls.append(_t.time() - t0)
    return min(reals), min(nulls), reals, nulls


def bench_exec(nc, in_maps, iters=10):
    """Device-resident repeated execution of the compiled kernel; returns
    per-call wall times (s) with inputs pre-staged on the 8 cores so only
    dispatch + device execution is measured."""
    import time as _t

    import jax
    import jax.numpy as jnp
    import mybir  # noqa: F401  # (ensure concourse paths set)
    from jax.experimental.shard_map import shard_map
    from jax.sharding import Mesh, PartitionSpec

    import concourse.mybir as mb
    from concourse import bass2jax as B

    B.install_neuronx_cc_hook()
    n_cores = len(in_maps)
    partition_name = (
        nc.partition_id_tensor.name if nc.partition_id_tensor else None
    )
    in_names, out_names, out_avals, zero_shapes = [], [], [], []
    for alloc in nc.m.functions[0].allocations:
        if not isinstance(alloc, mb.MemoryLocationSet):
            continue
        name = alloc.memorylocations[0].name
        if alloc.kind == "ExternalInput":
            if name != partition_name:
                in_names.append(name)
        elif alloc.kind == "ExternalOutput":
            shape = tuple(alloc.tensor_shape)
            dtype = mb.dt.np(alloc.dtype)
            out_names.append(name)
            out_avals.append(jax.core.ShapedArray(shape, dtype))
            zero_shapes.append((shape, dtype))
    n_params = len(in_names)
    all_in_names = list(in_names) + list(out_names)
    if partition_name is not None:
        all_in_names.append(partition_name)
    donate = tuple(range(n_params, n_params + len(out_names)))

    def _body(*args):
        operands = list(args)
        if partition_name is not None:
            operands.append(B.partition_id_tensor())
        outs = B._bass_exec_p.bind(
            *operands,
            out_avals=tuple(out_avals),
            in_names=tuple(all_in_names),
            out_names=tuple(out_names),
            lowering_input_output_aliases=(),
            sim_require_finite=True,
            sim_require_nnan=True,
            nc=nc,
        )
        return tuple(outs)

    devices = jax.devices()[:n_cores]
    mesh = Mesh(np.asarray(devices), ("core",))
    in_specs = (PartitionSpec("core"),) * (n_params + len(out_names))
    out_specs = (PartitionSpec("core"),) * len(out_names)
    fn = jax.jit(
        shard_map(
            _body, mesh=mesh, in_specs=in_specs, out_specs=out_specs,
            check_rep=False,
        ),
        donate_argnums=donate,
        keep_unused=True,
    )
    sh = jax.sharding.NamedSharding(mesh, PartitionSpec("core"))
    dev_in = [
        jax.device_put(
            np.concatenate([np.asarray(m[nm]) for m in in_maps], axis=0), sh
        )
        for nm in in_names
    ]

    def zeros():
        return [
            jax.device_put(
                jnp.zeros((n_cores * s[0],) + tuple(s[1:]), dt), sh
            )
            for (s, dt) in zero_shapes
        ]

    # warm up (compile + first exec)
    outs = fn(*dev_in, *zeros())
    jax.block_until_ready(outs)
    times = []
    for _ in range(iters):
        z = zeros()
        jax.block_until_ready(z)
        t0 = _t.perf_counter()
        outs = fn(*dev_in, *z)
        jax.block_until_ready(outs)
        times.append(_t.perf_counter() - t0)
    return times


def kernel(**inputs) -> np.ndarray:
    x = np.asarray(inputs["x"], dtype=np.float32)
    W = np.asarray(inputs["W"], dtype=np.float32)
    Es = [np.asarray(inputs[f"E{i}"], dtype=np.float32) for i in range(4)]
    bias = np.asarray(inputs["bias"], dtype=np.float32)
    ms = [np.asarray(inputs[f"m{i}"]).astype(bool) for i in range(4)]

    trace = bool(int(os.environ.get("HGRAPH_TRACE", "0")))
    out = _run(x, W, Es[1:], bias, ms[1:], N_CORES, trace=trace)

    f = W.shape[2]
    n = x.shape[1]
    # Safety net 1: hop 0 assumes m0 == I (structurally true for this module).
    if not np.array_equal(ms[0], np.eye(n, dtype=bool)):
        s0 = np.where(ms[0], Es[0], NEG)
        s0 = s0 - s0.max(axis=1, keepdims=True)
        p0 = np.exp(s0)
        a0 = p0 / p0.sum(axis=1, keepdims=True)
        h0 = np.einsum("bnf,fo->bno", x, W[0])
        out[:, :, 0:f] = np.einsum("ij,bjo->bio", a0, h0) + bias[None, None, :f]
    # Safety net 2: all-masked rows (softmax -> uniform; device would give NaN).
    for k in range(1, 4):
        empty = ~ms[k].any(axis=1)
        if empty.any():
            hk = np.einsum("bnf,fo->bno", x, W[k])
            unif = hk.mean(axis=1)  # [B, f]
            idx = np.where(empty)[0]
            out[:, idx, k * f:(k + 1) * f] = unif[:, None, :] + bias[None, None, k * f:(k + 1) * f]
    return out


def bench_pipelined(nc, in_maps, k=16):
    """Issue k executions back-to-back without host sync; returns total wall.
    If dispatches pipeline, slope vs k isolates device execution time."""
    import time as _t

    import jax
    import jax.numpy as jnp
    from jax.experimental.shard_map import shard_map
    from jax.sharding import Mesh, PartitionSpec

    import concourse.mybir as mb
    from concourse import bass2jax as B

    B.install_neuronx_cc_hook()
    n_cores = len(in_maps)
    partition_name = nc.partition_id_tensor.name if nc.partition_id_tensor else None
    in_names, out_names, out_avals, zero_shapes = [], [], [], []
    for alloc in nc.m.functions[0].allocations:
        if not isinstance(alloc, mb.MemoryLocationSet):
            continue
        name = alloc.memorylocations[0].name
        if alloc.kind == "ExternalInput":
            if name != partition_name:
                in_names.append(name)
        elif alloc.kind == "ExternalOutput":
            shape = tuple(alloc.tensor_shape)
            dtype = mb.dt.np(alloc.dtype)
            out_names.append(name)
            out_avals.append(jax.core.ShapedArray(shape, dtype))
            zero_shapes.append((shape, dtype))
    n_params = len(in_names)
    all_in_names = list(in_names) + list(out_names)
    if partition_name is not None:
        all_in_names.append(partition_name)
    donate = tuple(range(n_params, n_params + len(out_names)))

    def _body(*args):
        operands = list(args)
        if partition_name is not None:
            operands.append(B.partition_id_tensor())
        outs = B._bass_exec_p.bind(
            *operands,
            out_avals=tuple(out_avals),
            in_names=tuple(all_in_names),
            out_names=tuple(out_names),
            lowering_input_output_aliases=(),
            sim_require_finite=True,
            sim_require_nnan=True,
            nc=nc,
        )
        return tuple(outs)

    devices = jax.devices()[:n_cores]
    mesh = Mesh(np.asarray(devices), ("core",))
    in_specs = (PartitionSpec("core"),) * (n_params + len(out_names))
    out_specs = (PartitionSpec("core"),) * len(out_names)
    fn = jax.jit(
        shard_map(_body, mesh=mesh, in_specs=in_specs, out_specs=out_specs,
                  check_rep=False),
        donate_argnums=donate, keep_unused=True,
    )
    sh = jax.sharding.NamedSharding(mesh, PartitionSpec("core"))
    dev_in = [
        jax.device_put(
            np.concatenate([np.asarray(m[nm]) for m in in_maps], axis=0), sh
        )
        for nm in in_names
    ]

    def zeros():
        return [
            jax.device_put(jnp.zeros((n_cores * s[0],) + tuple(s[1:]), dt), sh)
            for (s, dt) in zero_shapes
        ]

    outs = fn(*dev_in, *zeros())
    jax.block_until_ready(outs)
    zs = [zeros() for _ in range(k)]
    for z in zs:
        jax.block_until_ready(z)
    t0 = _t.perf_counter()
    res = []
    for z in zs:
        res.append(fn(*dev_in, *z))
    jax.block_until_ready(res)
    return _t.perf_counter() - t0



# revision 20
# speedup vs baseline: 7.2771x; 2.2644x over previous
"""Trainium2 Bass kernel for nn_HGraphConv (4-hop masked-softmax graph conv).

Math per hop k:  out_k = softmax(where(m_k, E_k, NEG), axis=1) @ (x @ W_k)
Final:           concat(out_0..out_3, axis=2) + bias

Device strategy (data-parallel over batch B=64 across 8 cores, 8 batches/core):
  - P_k = exp(E_k) * m_k  computed in transposed [j, i] layout so 128x128
    slices are directly the matmul stationary operand (no on-chip transposes).
    Masked entries are exactly 0, matching softmax-with-NEG-fill exactly
    (no empty mask rows for this graph; checked on host with a numpy patch
    as fallback).
  - H_k[j, (b,f)] = x[b] @ W_k computed on device from x.T shards.
  - out[i, (b,f)] += P.T-tile @ H-tile accumulated over j in PSUM; the row
    sum Z[i] comes from one extra N=1 matmul vs a ones-vector that reuses
    the already-loaded stationary tile.
  - Eviction fuses the 1/Z softmax normalization (per-partition scalar) and
    the bias add (free-dim vector) in one scalar_tensor_tensor op:
        out = psum * (1/Z) + bias_rep   (valid because softmax rows sum to 1)
  - Hop 0 has m_0 = I => A_0 = I exactly, so out_0 = x @ W_0 + bias_0 and
    E_0/m_0 are never loaded (verified on host, numpy fallback otherwise).
"""

import os
import sys

import numpy as np

sys.path.insert(0, "/opt/trn_rl_repo")
sys.path.insert(0, "/opt/trn_rl_repo/concourse")

import concourse.bass as bass  # noqa: E402
import concourse.mybir as mybir  # noqa: E402
import concourse.tile as tile  # noqa: E402
import concourse.bass_utils as _bu  # noqa: E402
import concourse.bass2jax as _b2j  # noqa: E402
from concourse.bass_utils import run_bass_kernel_spmd  # noqa: E402

# ---------------------------------------------------------------------------
# Workaround for this walrus build: the TRN2 ISA has exactly one sync-wait
# slot per 64B instruction, and this compiler errors ("Too many sync wait
# commands") instead of splitting multi-wait instructions emitted by Tile.
# Split them ourselves at the BIR-JSON level: hoist all but one wait onto
# single-wait NoOps inserted right before the instruction on the same engine
# queue (queue waits execute in order, so this is semantically identical).
# ---------------------------------------------------------------------------
import json as _json  # noqa: E402


def _split_multi_waits_json(bir_json):
    if isinstance(bir_json, (bytes, bytearray)):
        m = _json.loads(bir_json.decode())
    else:
        m = _json.loads(bir_json)
    ctr = 0
    for fn in m["functions"]:
        for blk in fn["blocks"]:
            out = []
            for inst in blk["instructions"]:
                si = inst.get("sync_info")
                if si:
                    ws = si.get("on_wait") or []
                    if len(ws) > 1:
                        for w in ws[:-1]:
                            ctr += 1
                            out.append(
                                {
                                    "debug": inst.get("debug", 0),
                                    "engine": inst["engine"],
                                    "ins": [],
                                    "name": f"WX-{ctr}",
                                    "opcode": "NoOp",
                                    "outs": [],
                                    "text_hint": "split_wait",
                                    "sync_info": {
                                        "on_update": [],
                                        "on_wait": [w],
                                    },
                                }
                            )
                        si["on_wait"] = [ws[-1]]
                    us = si.get("on_update") or []
                    if len(us) > 1:
                        raise RuntimeError(
                            f"multi-update inst {inst['name']}: unsupported"
                        )
                out.append(inst)
            blk["instructions"] = out
    return _json.dumps(m).encode()


_orig_compile_bir_kernel = _bu.compile_bir_kernel.__wrapped__ if hasattr(
    _bu.compile_bir_kernel, "__wrapped__"
) else _bu.compile_bir_kernel


def _patched_compile_bir_kernel(bir_json, tmpdir, neff_name="file.neff"):
    return _orig_compile_bir_kernel(
        _split_multi_waits_json(bir_json), tmpdir, neff_name
    )


_bu.compile_bir_kernel = _patched_compile_bir_kernel
if hasattr(_b2j, "compile_bir_kernel"):
    _b2j.compile_bir_kernel = _patched_compile_bir_kernel

N_CORES = 8
B = 64
N = 1024
F = 128
HOPS = 4
NEG = -9.0e15

# filled by kernel() for test.py to read
last_run_info = {}


def build_nc(b_local: int, n: int, f: int = 128, reps: int = 1, variant: str = ""):
    """Build the per-core Bass module (software-pipelined).

    Math: out_k = P_k @ (x@W_k) / Z_k + bias_k with P_k = exp(folded E_k^T),
    Z_k = row sums via ones-matmul; hop 0 is the identity hop.

    Schedule per rep (steady state):
      [main1 (+Hb2, Hb0, et3-dma)] [exp2] [main2 (+Hb3)] [exp3]
      [main3-DR (+et1'/et2'-dma, Hb1')] [exp1']
    so every H-build, eviction, exp, and DMA hides under a main-matmul
    window.  Hop 3 runs in fp8e4 DoubleRow (K=256 per matmul); hops 1-2 in
    bf16.  PSUM stays fp32 throughout.
    """
    P = 128
    assert f == 128 and n % P == 0
    hc = b_local * f            # H columns per j-chunk
    assert hc == 1024, "pipelined build assumes b_local*f == 1024"
    n_half = hc // 512          # matmul column splits of H (=2)
    hw = 512                    # moving-operand width per matmul
    bg = hw // f                # batches per column split (=4)
    nch = n // P                # number of 128-row chunks (j and i)
    khops = HOPS - 1            # hops that need attention (1..3)

    nc = bass.Bass()
    fp32 = mybir.dt.float32
    bf16 = mybir.dt.bfloat16
    fp16 = mybir.dt.float16
    f8 = mybir.dt.float8e4
    DRM = mybir.MatmulPerfMode.DoubleRow
    xt_d = nc.dram_tensor("xt", [b_local, f, n], bf16, kind="ExternalInput")
    et_d = nc.dram_tensor("et", [khops, n, n], fp16, kind="ExternalInput")
    wc_d = nc.dram_tensor("wc", [f, HOPS * f], bf16, kind="ExternalInput")
    bias_d = nc.dram_tensor("bias", [HOPS * f], fp32, kind="ExternalInput")
    out_d = nc.dram_tensor("out", [HOPS, n_half, n, bg, f], bf16, kind="ExternalOutput")

    with tile.TileContext(nc) as tc:
        with (
            tc.tile_pool(name="const", bufs=1) as const,
            tc.tile_pool(name="et", bufs=2) as etp,
            tc.tile_pool(name="pt", bufs=2) as ptp,
            tc.tile_pool(name="pt8", bufs=2) as pt8p,
            tc.tile_pool(name="hh", bufs=2) as hhp,
            tc.tile_pool(name="hh2h", bufs=1) as hh2hp,
            tc.tile_pool(name="hh2l", bufs=1) as hh2lp,
            tc.tile_pool(name="hh3", bufs=1) as hh3p,
            tc.tile_pool(name="stage", bufs=4) as stp,
            tc.tile_pool(name="zi", bufs=4) as zip_,
            tc.tile_pool(name="psh", bufs=3, space="PSUM") as psh,
            tc.tile_pool(name="pso", bufs=4, space="PSUM") as pso,
            tc.tile_pool(name="psz", bufs=1, space="PSUM") as psz,
        ):
            # ---- constants (loaded once; outside the rep loop) ----
            xt = const.tile([P, b_local, n], bf16)
            nc.sync.dma_start(out=xt, in_=xt_d[:].rearrange("b f j -> f b j"))
            wc = const.tile([P, HOPS * f], bf16)
            nc.sync.dma_start(out=wc, in_=wc_d[:])
            ones = const.tile([P, 8], bf16)
            nc.vector.memset(ones, 1.0)
            ones8 = const.tile([P, 2, 1], f8)
            nc.vector.memset(ones8, 1.0)
            # bias replicated across partitions and the bg batches of a half
            br = const.tile([P, HOPS, bg, f], fp32)
            for k in range(HOPS):
                bsl = bias_d[k * f:(k + 1) * f]
                bcast = bass.AP(
                    tensor=bsl.tensor,
                    offset=bsl.offset,
                    ap=[[0, P], [0, bg], [1, f]],
                )
                nc.sync.dma_start(out=br[:, k], in_=bcast)

            # ---------------- emission helpers ----------------
            def emit_et_dmas(kk, name):
                """Chunked load of folded E_{kk+1}^T -> [128, nch, n] fp16."""
                t = etp.tile([P, nch, n], fp16, tag="et", name=name)
                for c in range(nch):
                    nc.sync.dma_start(
                        out=t[:, c], in_=et_d[kk, c * P:(c + 1) * P, :]
                    )
                return t

            def emit_exp(et, out_t):
                for c in range(0, nch, 2):
                    nc.scalar.activation(
                        out=out_t[:, c:c + 2, :],
                        in_=et[:, c:c + 2, :],
                        func=mybir.ActivationFunctionType.Exp,
                    )

            st0 = {}

            def hb_group(k, jc, h, hh_t):
                """One H-build psum tile: H_k rows jc, batches h*bg..h*bg+bg."""
                ps = psh.tile([P, hw], fp32, tag="psh")
                for bi in range(bg):
                    b = h * bg + bi
                    nc.tensor.matmul(
                        ps[:, bi * f:(bi + 1) * f],
                        xt[:, b, jc * P:(jc + 1) * P],
                        wc[:, k * f:(k + 1) * f],
                        start=True,
                        stop=True,
                    )
                brk = br[:, k].rearrange("p a b -> p (a b)")
                if k == 0:
                    # hop 0: A = I -> out0 = H0 + bias0, straight to DRAM
                    if jc not in st0:
                        st0[jc] = stp.tile(
                            [P, n_half, hw], bf16, tag="stage", name=f"st0_{jc}"
                        )
                    nc.vector.tensor_tensor(
                        out=st0[jc][:, h], in0=ps, in1=brk,
                        op=mybir.AluOpType.add,
                    )
                    if h == n_half - 1:
                        dst = out_d[0, :, jc * P:(jc + 1) * P].rearrange(
                            "h n b f -> n h b f"
                        )
                        nc.sync.dma_start(
                            out=dst,
                            in_=st0[jc].rearrange("p h (b f) -> p h b f", b=bg),
                        )
                        del st0[jc]
                elif k == 2:
                    # fp8 residual pair with bias folded: tmp = bf16(ps+b2);
                    # Hh = fp8(tmp); Hl = fp8(tmp - Hh)
                    tmp, h2h, h2l = hh_t
                    sl = (slice(None), jc, slice(h * hw, (h + 1) * hw))
                    nc.vector.tensor_tensor(
                        out=tmp[sl], in0=ps, in1=brk, op=mybir.AluOpType.add
                    )
                    nc.scalar.copy(out=h2h[sl], in_=tmp[sl])
                    nc.vector.tensor_tensor(
                        out=h2l[sl], in0=tmp[sl], in1=h2h[sl],
                        op=mybir.AluOpType.subtract,
                    )
                else:
                    # bias_k folded into H_k during the psum eviction
                    nc.vector.tensor_tensor(
                        out=hh_t[:, jc, h * hw:(h + 1) * hw],
                        in0=ps, in1=brk, op=mybir.AluOpType.add,
                    )

            def hb_thunks(k, hh_t):
                return [
                    (lambda jc=jc, h=h: hb_group(k, jc, h, hh_t))
                    for jc in range(nch)
                    for h in range(n_half)
                ]

            def main_hop(k, pt_t, rhs_sets, dr, extras):
                """Main attention matmuls for hop k; extras[i] emitted after
                ib block i.  rhs_sets: H tensors accumulated into the same
                PSUM group (fp8 hi/lo residual pair for hop 2)."""
                ei = 0
                per_ib = (len(extras) + nch - 1) // nch if extras else 0
                ns = len(rhs_sets)
                for ib in range(nch):
                    pz = psz.tile([P, 1], fp32, tag="psz")
                    pos = [
                        pso.tile([P, hw], fp32, tag="pso",
                                 name=f"pso_{k}_{ib}_{h2}")
                        for h2 in range(n_half)
                    ]
                    if dr:
                        for si, hh_t in enumerate(rhs_sets):
                            for c in range(nch // 2):
                                lhsT = pt_t[:, 2 * c:2 * c + 2,
                                            ib * P:(ib + 1) * P]
                                st_ = (si == 0 and c == 0)
                                sp_ = (si == ns - 1 and c == nch // 2 - 1)
                                for h in range(n_half):
                                    nc.tensor.matmul(
                                        pos[h],
                                        lhsT,
                                        hh_t[:, 2 * c:2 * c + 2,
                                             h * hw:(h + 1) * hw],
                                        start=st_,
                                        stop=sp_,
                                        perf_mode=DRM,
                                    )
                                if si == 0:
                                    nc.tensor.matmul(
                                        pz, lhsT, ones8[:],
                                        start=(c == 0),
                                        stop=(c == nch // 2 - 1),
                                        perf_mode=DRM,
                                    )
                    else:
                        hh_t = rhs_sets[0]
                        for jc in range(nch):
                            lhsT = pt_t[:, jc, ib * P:(ib + 1) * P]
                            st_, sp_ = (jc == 0), (jc == nch - 1)
                            for h in range(n_half):
                                nc.tensor.matmul(
                                    pos[h],
                                    lhsT,
                                    hh_t[:, jc, h * hw:(h + 1) * hw],
                                    start=st_,
                                    stop=sp_,
                                )
                            nc.tensor.matmul(
                                pz, lhsT, ones[:, 0:1], start=st_, stop=sp_
                            )
                    zinv = zip_.tile([P, 1], fp32, tag="zi")
                    nc.vector.reciprocal(out=zinv, in_=pz)
                    st = stp.tile([P, n_half, hw], bf16, tag="stage")
                    for h in range(n_half):
                        # bias folded into H; just scale by 1/Z (Act)
                        nc.scalar.activation(
                            out=st[:, h],
                            in_=pos[h],
                            func=mybir.ActivationFunctionType.Copy,
                            scale=zinv,
                        )
                    dst = out_d[k, :, ib * P:(ib + 1) * P].rearrange(
                        "h n b f -> n h b f"
                    )
                    nc.sync.dma_start(
                        out=dst, in_=st.rearrange("p h (b f) -> p h b f", b=bg)
                    )
                    for _ in range(per_ib):
                        if ei < len(extras):
                            extras[ei]()
                            ei += 1
                while ei < len(extras):
                    extras[ei]()
                    ei += 1

            # ---------------- pipelined rep loop ----------------
            # prologue for rep 0
            et1 = emit_et_dmas(0, "et1_r0")
            et2 = emit_et_dmas(1, "et2_r0")
            hh1 = hhp.tile([P, nch, hc], bf16, tag="hh", name="hh1_r0")
            for t in hb_thunks(1, hh1):
                t()
            pt1 = ptp.tile([P, nch, n], bf16, tag="pt", name="pt1_r0")
            emit_exp(et1, pt1)

            for rep in range(reps):
                # ---- hop 1 window: main1 + (Hb2 residual chain, et3 dma) ----
                tmp2 = hhp.tile([P, nch, hc], bf16, tag="hh", name=f"tmp2_r{rep}")
                h2h = hh2hp.tile([P, nch, hc], f8, tag="hh2h", name=f"h2h_r{rep}")
                h2l = hh2lp.tile([P, nch, hc], f8, tag="hh2l", name=f"h2l_r{rep}")
                et3_box = []

                def et3_thunk(rep=rep, box=et3_box):
                    box.append(emit_et_dmas(2, f"et3_r{rep}"))

                extras1 = []
                hb2 = hb_thunks(2, (tmp2, h2h, h2l))
                for i in range(nch):
                    extras1.extend(hb2[2 * i:2 * i + 2])
                    if i == 0:
                        extras1.append(et3_thunk)
                main_hop(1, pt1, [hh1], dr=False, extras=extras1)

                # ---- exp2 (fp8), hop 2 window: main2-DR + Hb3 ----
                pt2 = pt8p.tile([P, nch, n], f8, tag="pt8", name=f"pt2_r{rep}")
                emit_exp(et2, pt2)
                hh3 = hh3p.tile([P, nch, hc], f8, tag="hh3", name=f"hh3_r{rep}")
                extras2 = []
                hb3 = hb_thunks(3, hh3)
                hb0 = hb_thunks(0, None)
                for i in range(nch):
                    extras2.extend(hb3[2 * i:2 * i + 2])
                    extras2.extend(hb0[2 * i:2 * i + 2])
                main_hop(2, pt2, [h2h, h2l], dr=True, extras=extras2)

                # ---- exp3 (fp8), hop 3 window: main3-DR + next-rep prefetch ----
                pt3 = pt8p.tile([P, nch, n], f8, tag="pt8", name=f"pt3_r{rep}")
                emit_exp(et3_box[0], pt3)
                extras3 = []
                if rep < reps - 1:
                    hh1n = hhp.tile([P, nch, hc], bf16, tag="hh",
                                    name=f"hh1_r{rep + 1}")
                    nxt = []

                    def et12_thunk(rep=rep, box=nxt):
                        box.append(emit_et_dmas(0, f"et1_r{rep + 1}"))
                        box.append(emit_et_dmas(1, f"et2_r{rep + 1}"))

                    extras3 = [et12_thunk] + hb_thunks(1, hh1n)
                main_hop(3, pt3, [hh3], dr=True, extras=extras3)
                if rep < reps - 1:
                    et1, et2 = nxt[0], nxt[1]
                    hh1 = hh1n
                    pt1 = ptp.tile([P, nch, n], bf16, tag="pt",
                                   name=f"pt1_r{rep + 1}")
                    emit_exp(et1, pt1)
    return nc


_nc_cache = {}


def _get_nc(b_local, n, f):
    key = (b_local, n, f)
    if key not in _nc_cache:
        _nc_cache[key] = build_nc(b_local, n, f)
    return _nc_cache[key]


def _run(x, W, Es, bias, ms, n_cores, trace=False):
    """x:[B,N,F] W:[4,F,F] Es:[E1,E2,E3] ms:[m1,m2,m3] (hop-0 handled as identity)."""
    import ml_dtypes

    bf16 = ml_dtypes.bfloat16
    b, n, f = x.shape
    b_local = b // n_cores
    nc = _get_nc(b_local, n, f)

    # Fold mask into E (masked -> -65504 so device exp underflows to 0),
    # transpose to [j, i], and stage as fp16 (|E| ~ 5 so fp16 is exact to
    # ~5e-4 relative; halves the dominant DMA stream).
    et = np.stack(
        [np.where(m, e, np.float32(-65504.0)).T for e, m in zip(Es, ms)]
    ).astype(np.float16)
    wc = np.ascontiguousarray(
        np.concatenate([W[k] for k in range(HOPS)], axis=1).astype(bf16)
    )
    bias = np.ascontiguousarray(bias.astype(np.float32))

    in_maps = []
    for c in range(n_cores):
        xs = x[c * b_local:(c + 1) * b_local]          # [b_local, n, f]
        xts = np.ascontiguousarray(xs.transpose(0, 2, 1).astype(bf16))
        in_maps.append({"xt": xts, "et": et, "wc": wc, "bias": bias})

    last_run_info["nc"] = nc
    last_run_info["in_maps"] = in_maps
    res = run_bass_kernel_spmd(
        nc, in_maps, core_ids=list(range(n_cores)), trace=trace
    )
    last_run_info["exec_time_ns"] = res.exec_time_ns
    last_run_info["trace"] = res.instructions_and_trace

    out = np.empty((b, n, HOPS * f), dtype=np.float32)
    for c in range(n_cores):
        od = np.asarray(res.results[c]["out"]).astype(np.float32)
        nh, bg2 = od.shape[1], od.shape[3]   # [HOPS, n_half, n, bg, f]
        for k in range(HOPS):
            for h in range(nh):
                blo = c * b_local + h * bg2
                out[blo:blo + bg2, :, k * f:(k + 1) * f] = od[k, h].transpose(1, 0, 2)
    return out


def build_null_nc(b_local: int, n: int, f: int = 128):
    """Same external tensors as build_nc but ~no device work — used to
    subtract host/transfer/dispatch overhead when estimating HW exec time."""
    P = 128
    khops = HOPS - 1
    nc = bass.Bass()
    fp32 = mybir.dt.float32
    bf16 = mybir.dt.bfloat16
    fp16 = mybir.dt.float16
    nc.dram_tensor("xt", [b_local, f, n], bf16, kind="ExternalInput")
    nc.dram_tensor("et", [khops, n, n], fp16, kind="ExternalInput")
    wc_d = nc.dram_tensor("wc", [f, HOPS * f], bf16, kind="ExternalInput")
    nc.dram_tensor("bias", [HOPS * f], fp32, kind="ExternalInput")
    n_half = max(1, (b_local * f) // 512)
    bg = min(4, b_local)
    out_d = nc.dram_tensor("out", [HOPS, n_half, n, bg, f], bf16, kind="ExternalOutput")
    with tile.TileContext(nc) as tc:
        with tc.tile_pool(name="p", bufs=1) as pool:
            t = pool.tile([P, 8], bf16)
            nc.sync.dma_start(out=t, in_=wc_d[:, 0:8])
            nc.sync.dma_start(out=out_d[0, 0, 0:P, 0, 0:8], in_=t)
    return nc


def time_exec(iters=3):
    """Re-execute the last-run kernel and a null kernel; return
    (min_real_s, min_null_s). Uses identical input tensors so transfer and
    dispatch overhead cancels in the difference."""
    import time as _t

    nc = last_run_info["nc"]
    in_maps = last_run_info["in_maps"]
    n_cores = len(in_maps)
    reals, nulls = [], []
    for _ in range(iters):
        t0 = _t.time()
        run_bass_kernel_spmd(nc, in_maps, core_ids=list(range(n_cores)))
        reals.append(_t.time() - t0)
    b_local, f, n = in_maps[0]["xt"].shape
    nnc = build_null_nc(b_local, n, f)
    for _ in range(iters):
        t0 = _t.time()
        run_bass_kernel_spmd(nnc, in_maps, core_ids=list(range(n_cores)))
        nulls.append(_t.time() - t0)
    return min(reals), min(nulls), reals, nulls


def bench_exec(nc, in_maps, iters=10):
    """Device-resident repeated execution of the compiled kernel; returns
    per-call wall times (s) with inputs pre-staged on the 8 cores so only
    dispatch + device execution is measured."""
    import time as _t

    import jax
    import jax.numpy as jnp
    import mybir  # noqa: F401  # (ensure concourse paths set)
    from jax.experimental.shard_map import shard_map
    from jax.sharding import Mesh, PartitionSpec

    import concourse.mybir as mb
    from concourse import bass2jax as B

    B.install_neuronx_cc_hook()
    n_cores = len(in_maps)
    partition_name = (
        nc.partition_id_tensor.name if nc.partition_id_tensor else None
    )
    in_names, out_names, out_avals, zero_shapes = [], [], [], []
    for alloc in nc.m.functions[0].allocations:
        if not isinstance(alloc, mb.MemoryLocationSet):
            continue
        name = alloc.memorylocations[0].name
        if alloc.kind == "ExternalInput":
            if name != partition_name:
                in_names.append(name)
        elif alloc.kind == "ExternalOutput":
            shape = tuple(alloc.tensor_shape)
            dtype = mb.dt.np(alloc.dtype)
            out_names.append(name)
            out_avals.append(jax.core.ShapedArray(shape, dtype))
            zero_shapes.append((shape, dtype))
    n_params = len(in_names)
    all_in_names = list(in_names) + list(out_names)
    if partition_name is not None:
        all_in_names.append(partition_name)
    donate = tuple(range(n_params, n_params + len(out_names)))

    def _body(*args):
        operands = list(args)
        if partition_name is not None:
            operands.append(B.partition_id_tensor())
        outs = B._bass_exec_p.bind(
            *operands,
            out_avals=tuple(out_avals),
            in_names=tuple(all_in_names),
            out_names=tuple(out_names),
            lowering_input_output_aliases=(),
            sim_require_finite=True,
            sim_require_nnan=True,
            nc=nc,
        )
        return tuple(outs)

    devices = jax.devices()[:n_cores]
    mesh = Mesh(np.asarray(devices), ("core",))
    in_specs = (PartitionSpec("core"),) * (n_params + len(out_names))
    out_specs = (PartitionSpec("core"),) * len(out_names)
    fn = jax.jit(
        shard_map(
            _body, mesh=mesh, in_specs=in_specs, out_specs=out_specs,
            check_rep=False,
        ),
        donate_argnums=donate,
        keep_unused=True,
    )
    sh = jax.sharding.NamedSharding(mesh, PartitionSpec("core"))
    dev_in = [
        jax.device_put(
            np.concatenate([np.asarray(m[nm]) for m in in_maps], axis=0), sh
        )
        for nm in in_names
    ]

    def zeros():
        return [
            jax.device_put(
                jnp.zeros((n_cores * s[0],) + tuple(s[1:]), dt), sh
            )
            for (s, dt) in zero_shapes
        ]

    # warm up (compile + first exec)
    outs = fn(*dev_in, *zeros())
    jax.block_until_ready(outs)
    times = []
    for _ in range(iters):
        z = zeros()
        jax.block_until_ready(z)
        t0 = _t.perf_counter()
        outs = fn(*dev_in, *z)
        jax.block_until_ready(outs)
        times.append(_t.perf_counter() - t0)
    return times


def kernel(**inputs) -> np.ndarray:
    x = np.asarray(inputs["x"], dtype=np.float32)
    W = np.asarray(inputs["W"], dtype=np.float32)
    Es = [np.asarray(inputs[f"E{i}"], dtype=np.float32) for i in range(4)]
    bias = np.asarray(inputs["bias"], dtype=np.float32)
    ms = [np.asarray(inputs[f"m{i}"]).astype(bool) for i in range(4)]

    trace = bool(int(os.environ.get("HGRAPH_TRACE", "0")))
    out = _run(x, W, Es[1:], bias, ms[1:], N_CORES, trace=trace)

    f = W.shape[2]
    n = x.shape[1]
    # Safety net 1: hop 0 assumes m0 == I (structurally true for this module).
    if not np.array_equal(ms[0], np.eye(n, dtype=bool)):
        s0 = np.where(ms[0], Es[0], NEG)
        s0 = s0 - s0.max(axis=1, keepdims=True)
        p0 = np.exp(s0)
        a0 = p0 / p0.sum(axis=1, keepdims=True)
        h0 = np.einsum("bnf,fo->bno", x, W[0])
        out[:, :, 0:f] = np.einsum("ij,bjo->bio", a0, h0) + bias[None, None, :f]
    # Safety net 2: all-masked rows (softmax -> uniform; device would give NaN).
    for k in range(1, 4):
        empty = ~ms[k].any(axis=1)
        if empty.any():
            hk = np.einsum("bnf,fo->bno", x, W[k])
            unif = hk.mean(axis=1)  # [B, f]
            idx = np.where(empty)[0]
            out[:, idx, k * f:(k + 1) * f] = unif[:, None, :] + bias[None, None, k * f:(k + 1) * f]
    return out


def bench_pipelined(nc, in_maps, k=16):
    """Issue k executions back-to-back without host sync; returns total wall.
    If dispatches pipeline, slope vs k isolates device execution time."""
    import time as _t

    import jax
    import jax.numpy as jnp
    from jax.experimental.shard_map import shard_map
    from jax.sharding import Mesh, PartitionSpec

    import concourse.mybir as mb
    from concourse import bass2jax as B

    B.install_neuronx_cc_hook()
    n_cores = len(in_maps)
    partition_name = nc.partition_id_tensor.name if nc.partition_id_tensor else None
    in_names, out_names, out_avals, zero_shapes = [], [], [], []
    for alloc in nc.m.functions[0].allocations:
        if not isinstance(alloc, mb.MemoryLocationSet):
            continue
        name = alloc.memorylocations[0].name
        if alloc.kind == "ExternalInput":
            if name != partition_name:
                in_names.append(name)
        elif alloc.kind == "ExternalOutput":
            shape = tuple(alloc.tensor_shape)
            dtype = mb.dt.np(alloc.dtype)
            out_names.append(name)
            out_avals.append(jax.core.ShapedArray(shape, dtype))
            zero_shapes.append((shape, dtype))
    n_params = len(in_names)
    all_in_names = list(in_names) + list(out_names)
    if partition_name is not None:
        all_in_names.append(partition_name)
    donate = tuple(range(n_params, n_params + len(out_names)))

    def _body(*args):
        operands = list(args)
        if partition_name is not None:
            operands.append(B.partition_id_tensor())
        outs = B._bass_exec_p.bind(
            *operands,
            out_avals=tuple(out_avals),
            in_names=tuple(all_in_names),
            out_names=tuple(out_names),
            lowering_input_output_aliases=(),
            sim_require_finite=True,
            sim_require_nnan=True,
            nc=nc,
        )
        return tuple(outs)

    devices = jax.devices()[:n_cores]
    mesh = Mesh(np.asarray(devices), ("core",))
    in_specs = (PartitionSpec("core"),) * (n_params + len(out_names))
    out_specs = (PartitionSpec("core"),) * len(out_names)
    fn = jax.jit(
        shard_map(_body, mesh=mesh, in_specs=in_specs, out_specs=out_specs,
                  check_rep=False),
        donate_argnums=donate, keep_unused=True,
    )
    sh = jax.sharding.NamedSharding(mesh, PartitionSpec("core"))
    dev_in = [
        jax.device_put(
            np.concatenate([np.asarray(m[nm]) for m in in_maps], axis=0), sh
        )
        for nm in in_names
    ]

    def zeros():
        return [
            jax.device_put(jnp.zeros((n_cores * s[0],) + tuple(s[1:]), dt), sh)
            for (s, dt) in zero_shapes
        ]

    outs = fn(*dev_in, *zeros())
    jax.block_until_ready(outs)
    zs = [zeros() for _ in range(k)]
    for z in zs:
        jax.block_until_ready(z)
    t0 = _t.perf_counter()
    res = []
    for z in zs:
        res.append(fn(*dev_in, *z))
    jax.block_until_ready(res)
    return _t.perf_counter() - t0



# revision 21
# speedup vs baseline: 12.5383x; 1.7230x over previous
"""Trainium2 Bass kernel for nn_HGraphConv (4-hop masked-softmax graph conv).

Math per hop k:  out_k = softmax(where(m_k, E_k, NEG), axis=1) @ (x @ W_k)
Final:           concat(out_0..out_3, axis=2) + bias

Device strategy (data-parallel over batch B=64 across 8 cores, 8 batches/core):
  - E_k staged host-side as fp16 with the mask folded in (masked = -65504,
    exp -> exactly 0) in transposed [j, i] layout so 128x128 slices are
    directly the matmul stationary operand; x/W staged bf16; output bf16,
    upcast on host.
  - Hop dtypes: hop1 bf16; hop2 P=fp8e4 + H as an fp8 hi/lo residual pair
    (so the H path keeps ~bf16 accuracy while both matmul operands are fp8);
    hop3 P,H fp8e4.  fp8 hops use DoubleRow perf mode (K=256/matmul, ~4x
    bf16 throughput); PSUM accumulates fp32.  Softmax renormalization by the
    consistently-computed Z cancels most of P's quantization error.
  - Z_k row sums via ones-matmuls sharing the loaded stationary tiles; the
    final evict applies 1/Z (per-partition scalar on Act); bias rides the
    H eviction (valid since P rows /Z sum to exactly 1).
  - Software pipeline: each hop's main-matmul window hides the next hop's
    H-build + eviction, exp, and E-prefetch DMAs; the hop-3 window prefetches
    the NEXT rep's hop-1 state, so consecutive reps overlap seamlessly.
  - Hop 0 has m_0 = I => out_0 = x @ W_0 + bias_0 straight from the H-build
    PSUM (E_0/m_0 never loaded; host numpy fallback if m_0 != I).
  - Host safety nets: m0 != I, and all-masked rows (device would yield NaN)
    are patched on host; both no-ops for this graph.
"""

import os
import sys

import numpy as np

sys.path.insert(0, "/opt/trn_rl_repo")
sys.path.insert(0, "/opt/trn_rl_repo/concourse")

import concourse.bass as bass  # noqa: E402
import concourse.mybir as mybir  # noqa: E402
import concourse.tile as tile  # noqa: E402
import concourse.bass_utils as _bu  # noqa: E402
import concourse.bass2jax as _b2j  # noqa: E402
from concourse.bass_utils import run_bass_kernel_spmd  # noqa: E402

# ---------------------------------------------------------------------------
# Workaround for this walrus build: the TRN2 ISA has exactly one sync-wait
# slot per 64B instruction, and this compiler errors ("Too many sync wait
# commands") instead of splitting multi-wait instructions emitted by Tile.
# Split them ourselves at the BIR-JSON level: hoist all but one wait onto
# single-wait NoOps inserted right before the instruction on the same engine
# queue (queue waits execute in order, so this is semantically identical).
# ---------------------------------------------------------------------------
import json as _json  # noqa: E402


def _split_multi_waits_json(bir_json):
    if isinstance(bir_json, (bytes, bytearray)):
        m = _json.loads(bir_json.decode())
    else:
        m = _json.loads(bir_json)
    ctr = 0
    for fn in m["functions"]:
        for blk in fn["blocks"]:
            out = []
            for inst in blk["instructions"]:
                si = inst.get("sync_info")
                if si:
                    ws = si.get("on_wait") or []
                    if len(ws) > 1:
                        for w in ws[:-1]:
                            ctr += 1
                            out.append(
                                {
                                    "debug": inst.get("debug", 0),
                                    "engine": inst["engine"],
                                    "ins": [],
                                    "name": f"WX-{ctr}",
                                    "opcode": "NoOp",
                                    "outs": [],
                                    "text_hint": "split_wait",
                                    "sync_info": {
                                        "on_update": [],
                                        "on_wait": [w],
                                    },
                                }
                            )
                        si["on_wait"] = [ws[-1]]
                    us = si.get("on_update") or []
                    if len(us) > 1:
                        raise RuntimeError(
                            f"multi-update inst {inst['name']}: unsupported"
                        )
                out.append(inst)
            blk["instructions"] = out
    return _json.dumps(m).encode()


_orig_compile_bir_kernel = _bu.compile_bir_kernel.__wrapped__ if hasattr(
    _bu.compile_bir_kernel, "__wrapped__"
) else _bu.compile_bir_kernel


def _patched_compile_bir_kernel(bir_json, tmpdir, neff_name="file.neff"):
    return _orig_compile_bir_kernel(
        _split_multi_waits_json(bir_json), tmpdir, neff_name
    )


_bu.compile_bir_kernel = _patched_compile_bir_kernel
if hasattr(_b2j, "compile_bir_kernel"):
    _b2j.compile_bir_kernel = _patched_compile_bir_kernel

N_CORES = 8
B = 64
N = 1024
F = 128
HOPS = 4
NEG = -9.0e15

# filled by kernel() for test.py to read
last_run_info = {}


def build_nc(b_local: int, n: int, f: int = 128, reps: int = 1, variant: str = ""):
    """Build the per-core Bass module (software-pipelined).

    Math: out_k = P_k @ (x@W_k) / Z_k + bias_k with P_k = exp(folded E_k^T),
    Z_k = row sums via ones-matmul; hop 0 is the identity hop.

    Schedule per rep (steady state):
      [main1 (+Hb2, Hb0, et3-dma)] [exp2] [main2 (+Hb3)] [exp3]
      [main3-DR (+et1'/et2'-dma, Hb1')] [exp1']
    so every H-build, eviction, exp, and DMA hides under a main-matmul
    window.  Hop 3 runs in fp8e4 DoubleRow (K=256 per matmul); hops 1-2 in
    bf16.  PSUM stays fp32 throughout.
    """
    P = 128
    assert f == 128 and n % P == 0
    hc = b_local * f            # H columns per j-chunk
    assert hc == 1024, "pipelined build assumes b_local*f == 1024"
    n_half = hc // 512          # matmul column splits of H (=2)
    hw = 512                    # moving-operand width per matmul
    bg = hw // f                # batches per column split (=4)
    nch = n // P                # number of 128-row chunks (j and i)
    khops = HOPS - 1            # hops that need attention (1..3)

    nc = bass.Bass()
    fp32 = mybir.dt.float32
    bf16 = mybir.dt.bfloat16
    fp16 = mybir.dt.float16
    f8 = mybir.dt.float8e4
    DRM = mybir.MatmulPerfMode.DoubleRow
    xt_d = nc.dram_tensor("xt", [b_local, f, n], bf16, kind="ExternalInput")
    et_d = nc.dram_tensor("et", [khops, n, n], fp16, kind="ExternalInput")
    wc_d = nc.dram_tensor("wc", [f, HOPS * f], bf16, kind="ExternalInput")
    bias_d = nc.dram_tensor("bias", [HOPS * f], fp32, kind="ExternalInput")
    out_d = nc.dram_tensor("out", [HOPS, n_half, n, bg, f], bf16, kind="ExternalOutput")

    with tile.TileContext(nc) as tc:
        with (
            tc.tile_pool(name="const", bufs=1) as const,
            tc.tile_pool(name="et", bufs=2) as etp,
            tc.tile_pool(name="pt", bufs=2) as ptp,
            tc.tile_pool(name="pt8", bufs=2) as pt8p,
            tc.tile_pool(name="hh", bufs=2) as hhp,
            tc.tile_pool(name="hh2h", bufs=1) as hh2hp,
            tc.tile_pool(name="hh2l", bufs=1) as hh2lp,
            tc.tile_pool(name="hh3", bufs=1) as hh3p,
            tc.tile_pool(name="stage", bufs=4) as stp,
            tc.tile_pool(name="zi", bufs=4) as zip_,
            tc.tile_pool(name="psh", bufs=3, space="PSUM") as psh,
            tc.tile_pool(name="pso", bufs=4, space="PSUM") as pso,
            tc.tile_pool(name="psz", bufs=1, space="PSUM") as psz,
        ):
            # ---- constants (loaded once; outside the rep loop) ----
            xt = const.tile([P, b_local, n], bf16)
            nc.sync.dma_start(out=xt, in_=xt_d[:].rearrange("b f j -> f b j"))
            wc = const.tile([P, HOPS * f], bf16)
            nc.sync.dma_start(out=wc, in_=wc_d[:])
            ones = const.tile([P, 8], bf16)
            nc.vector.memset(ones, 1.0)
            ones8 = const.tile([P, 2, 1], f8)
            nc.vector.memset(ones8, 1.0)
            # bias replicated across partitions and the bg batches of a half
            br = const.tile([P, HOPS, bg, f], fp32)
            for k in range(HOPS):
                bsl = bias_d[k * f:(k + 1) * f]
                bcast = bass.AP(
                    tensor=bsl.tensor,
                    offset=bsl.offset,
                    ap=[[0, P], [0, bg], [1, f]],
                )
                nc.sync.dma_start(out=br[:, k], in_=bcast)

            # ---------------- emission helpers ----------------
            def emit_et_dmas(kk, name):
                """Chunked load of folded E_{kk+1}^T -> [128, nch, n] fp16."""
                t = etp.tile([P, nch, n], fp16, tag="et", name=name)
                for c in range(nch):
                    nc.sync.dma_start(
                        out=t[:, c], in_=et_d[kk, c * P:(c + 1) * P, :]
                    )
                return t

            def emit_exp(et, out_t):
                for c in range(0, nch, 2):
                    nc.scalar.activation(
                        out=out_t[:, c:c + 2, :],
                        in_=et[:, c:c + 2, :],
                        func=mybir.ActivationFunctionType.Exp,
                    )

            st0 = {}

            def hb_group(k, jc, h, hh_t):
                """One H-build psum tile: H_k rows jc, batches h*bg..h*bg+bg."""
                ps = psh.tile([P, hw], fp32, tag="psh")
                for bi in range(bg):
                    b = h * bg + bi
                    nc.tensor.matmul(
                        ps[:, bi * f:(bi + 1) * f],
                        xt[:, b, jc * P:(jc + 1) * P],
                        wc[:, k * f:(k + 1) * f],
                        start=True,
                        stop=True,
                    )
                brk = br[:, k].rearrange("p a b -> p (a b)")
                if k == 0:
                    # hop 0: A = I -> out0 = H0 + bias0, straight to DRAM
                    if jc not in st0:
                        st0[jc] = stp.tile(
                            [P, n_half, hw], bf16, tag="stage", name=f"st0_{jc}"
                        )
                    nc.vector.tensor_tensor(
                        out=st0[jc][:, h], in0=ps, in1=brk,
                        op=mybir.AluOpType.add,
                    )
                    if h == n_half - 1:
                        dst = out_d[0, :, jc * P:(jc + 1) * P].rearrange(
                            "h n b f -> n h b f"
                        )
                        nc.sync.dma_start(
                            out=dst,
                            in_=st0[jc].rearrange("p h (b f) -> p h b f", b=bg),
                        )
                        del st0[jc]
                elif k == 2:
                    # fp8 residual pair with bias folded: tmp = bf16(ps+b2);
                    # Hh = fp8(tmp); Hl = fp8(tmp - Hh)
                    tmp, h2h, h2l = hh_t
                    sl = (slice(None), jc, slice(h * hw, (h + 1) * hw))
                    nc.vector.tensor_tensor(
                        out=tmp[sl], in0=ps, in1=brk, op=mybir.AluOpType.add
                    )
                    nc.scalar.copy(out=h2h[sl], in_=tmp[sl])
                    nc.vector.tensor_tensor(
                        out=h2l[sl], in0=tmp[sl], in1=h2h[sl],
                        op=mybir.AluOpType.subtract,
                    )
                else:
                    # bias_k folded into H_k during the psum eviction
                    nc.vector.tensor_tensor(
                        out=hh_t[:, jc, h * hw:(h + 1) * hw],
                        in0=ps, in1=brk, op=mybir.AluOpType.add,
                    )

            def hb_thunks(k, hh_t):
                return [
                    (lambda jc=jc, h=h: hb_group(k, jc, h, hh_t))
                    for jc in range(nch)
                    for h in range(n_half)
                ]

            def main_hop(k, pt_t, rhs_sets, dr, extras):
                """Main attention matmuls for hop k; extras[i] emitted after
                ib block i.  rhs_sets: H tensors accumulated into the same
                PSUM group (fp8 hi/lo residual pair for hop 2)."""
                ei = 0
                per_ib = (len(extras) + nch - 1) // nch if extras else 0
                ns = len(rhs_sets)
                for ib in range(nch):
                    pz = psz.tile([P, 1], fp32, tag="psz")
                    pos = [
                        pso.tile([P, hw], fp32, tag="pso",
                                 name=f"pso_{k}_{ib}_{h2}")
                        for h2 in range(n_half)
                    ]
                    if dr:
                        for si, hh_t in enumerate(rhs_sets):
                            for c in range(nch // 2):
                                lhsT = pt_t[:, 2 * c:2 * c + 2,
                                            ib * P:(ib + 1) * P]
                                st_ = (si == 0 and c == 0)
                                sp_ = (si == ns - 1 and c == nch // 2 - 1)
                                for h in range(n_half):
                                    nc.tensor.matmul(
                                        pos[h],
                                        lhsT,
                                        hh_t[:, 2 * c:2 * c + 2,
                                             h * hw:(h + 1) * hw],
                                        start=st_,
                                        stop=sp_,
                                        perf_mode=DRM,
                                    )
                                if si == 0:
                                    nc.tensor.matmul(
                                        pz, lhsT, ones8[:],
                                        start=(c == 0),
                                        stop=(c == nch // 2 - 1),
                                        perf_mode=DRM,
                                    )
                    else:
                        hh_t = rhs_sets[0]
                        for jc in range(nch):
                            lhsT = pt_t[:, jc, ib * P:(ib + 1) * P]
                            st_, sp_ = (jc == 0), (jc == nch - 1)
                            for h in range(n_half):
                                nc.tensor.matmul(
                                    pos[h],
                                    lhsT,
                                    hh_t[:, jc, h * hw:(h + 1) * hw],
                                    start=st_,
                                    stop=sp_,
                                )
                            nc.tensor.matmul(
                                pz, lhsT, ones[:, 0:1], start=st_, stop=sp_
                            )
                    zinv = zip_.tile([P, 1], fp32, tag="zi")
                    nc.vector.reciprocal(out=zinv, in_=pz)
                    st = stp.tile([P, n_half, hw], bf16, tag="stage")
                    for h in range(n_half):
                        # bias folded into H; just scale by 1/Z (Act)
                        nc.scalar.activation(
                            out=st[:, h],
                            in_=pos[h],
                            func=mybir.ActivationFunctionType.Copy,
                            scale=zinv,
                        )
                    dst = out_d[k, :, ib * P:(ib + 1) * P].rearrange(
                        "h n b f -> n h b f"
                    )
                    nc.sync.dma_start(
                        out=dst, in_=st.rearrange("p h (b f) -> p h b f", b=bg)
                    )
                    for _ in range(per_ib):
                        if ei < len(extras):
                            extras[ei]()
                            ei += 1
                while ei < len(extras):
                    extras[ei]()
                    ei += 1

            # ---------------- pipelined rep loop ----------------
            # prologue for rep 0
            et1 = emit_et_dmas(0, "et1_r0")
            et2 = emit_et_dmas(1, "et2_r0")
            hh1 = hhp.tile([P, nch, hc], bf16, tag="hh", name="hh1_r0")
            for t in hb_thunks(1, hh1):
                t()
            pt1 = ptp.tile([P, nch, n], bf16, tag="pt", name="pt1_r0")
            emit_exp(et1, pt1)

            for rep in range(reps):
                # ---- hop 1 window: main1 + (Hb2 residual chain, et3 dma) ----
                tmp2 = hhp.tile([P, nch, hc], bf16, tag="hh", name=f"tmp2_r{rep}")
                h2h = hh2hp.tile([P, nch, hc], f8, tag="hh2h", name=f"h2h_r{rep}")
                h2l = hh2lp.tile([P, nch, hc], f8, tag="hh2l", name=f"h2l_r{rep}")
                et3_box = []

                def et3_thunk(rep=rep, box=et3_box):
                    box.append(emit_et_dmas(2, f"et3_r{rep}"))

                extras1 = []
                hb2 = hb_thunks(2, (tmp2, h2h, h2l))
                for i in range(nch):
                    extras1.extend(hb2[2 * i:2 * i + 2])
                    if i == 0:
                        extras1.append(et3_thunk)
                main_hop(1, pt1, [hh1], dr=False, extras=extras1)

                # ---- exp2 (fp8), hop 2 window: main2-DR + Hb3 ----
                pt2 = pt8p.tile([P, nch, n], f8, tag="pt8", name=f"pt2_r{rep}")
                emit_exp(et2, pt2)
                hh3 = hh3p.tile([P, nch, hc], f8, tag="hh3", name=f"hh3_r{rep}")
                extras2 = []
                hb3 = hb_thunks(3, hh3)
                hb0 = hb_thunks(0, None)
                for i in range(nch):
                    extras2.extend(hb3[2 * i:2 * i + 2])
                    extras2.extend(hb0[2 * i:2 * i + 2])
                main_hop(2, pt2, [h2h, h2l], dr=True, extras=extras2)

                # ---- exp3 (fp8), hop 3 window: main3-DR + next-rep prefetch ----
                pt3 = pt8p.tile([P, nch, n], f8, tag="pt8", name=f"pt3_r{rep}")
                emit_exp(et3_box[0], pt3)
                extras3 = []
                if rep < reps - 1:
                    hh1n = hhp.tile([P, nch, hc], bf16, tag="hh",
                                    name=f"hh1_r{rep + 1}")
                    nxt = []

                    def et12_thunk(rep=rep, box=nxt):
                        box.append(emit_et_dmas(0, f"et1_r{rep + 1}"))
                        box.append(emit_et_dmas(1, f"et2_r{rep + 1}"))

                    extras3 = [et12_thunk] + hb_thunks(1, hh1n)
                main_hop(3, pt3, [hh3], dr=True, extras=extras3)
                if rep < reps - 1:
                    et1, et2 = nxt[0], nxt[1]
                    hh1 = hh1n
                    pt1 = ptp.tile([P, nch, n], bf16, tag="pt",
                                   name=f"pt1_r{rep + 1}")
                    emit_exp(et1, pt1)
    return nc


_nc_cache = {}


def _get_nc(b_local, n, f):
    key = (b_local, n, f)
    if key not in _nc_cache:
        _nc_cache[key] = build_nc(b_local, n, f)
    return _nc_cache[key]


def _run(x, W, Es, bias, ms, n_cores, trace=False):
    """x:[B,N,F] W:[4,F,F] Es:[E1,E2,E3] ms:[m1,m2,m3] (hop-0 handled as identity)."""
    import ml_dtypes

    bf16 = ml_dtypes.bfloat16
    b, n, f = x.shape
    b_local = b // n_cores
    nc = _get_nc(b_local, n, f)

    # Fold mask into E (masked -> -65504 so device exp underflows to 0),
    # transpose to [j, i], and stage as fp16 (|E| ~ 5 so fp16 is exact to
    # ~5e-4 relative; halves the dominant DMA stream).
    et = np.stack(
        [np.where(m, e, np.float32(-65504.0)).T for e, m in zip(Es, ms)]
    ).astype(np.float16)
    wc = np.ascontiguousarray(
        np.concatenate([W[k] for k in range(HOPS)], axis=1).astype(bf16)
    )
    bias = np.ascontiguousarray(bias.astype(np.float32))

    in_maps = []
    for c in range(n_cores):
        xs = x[c * b_local:(c + 1) * b_local]          # [b_local, n, f]
        xts = np.ascontiguousarray(xs.transpose(0, 2, 1).astype(bf16))
        in_maps.append({"xt": xts, "et": et, "wc": wc, "bias": bias})

    last_run_info["nc"] = nc
    last_run_info["in_maps"] = in_maps
    res = run_bass_kernel_spmd(
        nc, in_maps, core_ids=list(range(n_cores)), trace=trace
    )
    last_run_info["exec_time_ns"] = res.exec_time_ns
    last_run_info["trace"] = res.instructions_and_trace

    out = np.empty((b, n, HOPS * f), dtype=np.float32)
    for c in range(n_cores):
        od = np.asarray(res.results[c]["out"]).astype(np.float32)
        nh, bg2 = od.shape[1], od.shape[3]   # [HOPS, n_half, n, bg, f]
        for k in range(HOPS):
            for h in range(nh):
                blo = c * b_local + h * bg2
                out[blo:blo + bg2, :, k * f:(k + 1) * f] = od[k, h].transpose(1, 0, 2)
    return out


def build_null_nc(b_local: int, n: int, f: int = 128):
    """Same external tensors as build_nc but ~no device work — used to
    subtract host/transfer/dispatch overhead when estimating HW exec time."""
    P = 128
    khops = HOPS - 1
    nc = bass.Bass()
    fp32 = mybir.dt.float32
    bf16 = mybir.dt.bfloat16
    fp16 = mybir.dt.float16
    nc.dram_tensor("xt", [b_local, f, n], bf16, kind="ExternalInput")
    nc.dram_tensor("et", [khops, n, n], fp16, kind="ExternalInput")
    wc_d = nc.dram_tensor("wc", [f, HOPS * f], bf16, kind="ExternalInput")
    nc.dram_tensor("bias", [HOPS * f], fp32, kind="ExternalInput")
    n_half = max(1, (b_local * f) // 512)
    bg = min(4, b_local)
    out_d = nc.dram_tensor("out", [HOPS, n_half, n, bg, f], bf16, kind="ExternalOutput")
    with tile.TileContext(nc) as tc:
        with tc.tile_pool(name="p", bufs=1) as pool:
            t = pool.tile([P, 8], bf16)
            nc.sync.dma_start(out=t, in_=wc_d[:, 0:8])
            nc.sync.dma_start(out=out_d[0, 0, 0:P, 0, 0:8], in_=t)
    return nc


def time_exec(iters=3):
    """Re-execute the last-run kernel and a null kernel; return
    (min_real_s, min_null_s). Uses identical input tensors so transfer and
    dispatch overhead cancels in the difference."""
    import time as _t

    nc = last_run_info["nc"]
    in_maps = last_run_info["in_maps"]
    n_cores = len(in_maps)
    reals, nulls = [], []
    for _ in range(iters):
        t0 = _t.time()
        run_bass_kernel_spmd(nc, in_maps, core_ids=list(range(n_cores)))
        reals.append(_t.time() - t0)
    b_local, f, n = in_maps[0]["xt"].shape
    nnc = build_null_nc(b_local, n, f)
    for _ in range(iters):
        t0 = _t.time()
        run_bass_kernel_spmd(nnc, in_maps, core_ids=list(range(n_cores)))
        nulls.append(_t.time() - t0)
    return min(reals), min(nulls), reals, nulls


def bench_exec(nc, in_maps, iters=10):
    """Device-resident repeated execution of the compiled kernel; returns
    per-call wall times (s) with inputs pre-staged on the 8 cores so only
    dispatch + device execution is measured."""
    import time as _t

    import jax
    import jax.numpy as jnp
    import mybir  # noqa: F401  # (ensure concourse paths set)
    from jax.experimental.shard_map import shard_map
    from jax.sharding import Mesh, PartitionSpec

    import concourse.mybir as mb
    from concourse import bass2jax as B

    B.install_neuronx_cc_hook()
    n_cores = len(in_maps)
    partition_name = (
        nc.partition_id_tensor.name if nc.partition_id_tensor else None
    )
    in_names, out_names, out_avals, zero_shapes = [], [], [], []
    for alloc in nc.m.functions[0].allocations:
        if not isinstance(alloc, mb.MemoryLocationSet):
            continue
        name = alloc.memorylocations[0].name
        if alloc.kind == "ExternalInput":
            if name != partition_name:
                in_names.append(name)
        elif alloc.kind == "ExternalOutput":
            shape = tuple(alloc.tensor_shape)
            dtype = mb.dt.np(alloc.dtype)
            out_names.append(name)
            out_avals.append(jax.core.ShapedArray(shape, dtype))
            zero_shapes.append((shape, dtype))
    n_params = len(in_names)
    all_in_names = list(in_names) + list(out_names)
    if partition_name is not None:
        all_in_names.append(partition_name)
    donate = tuple(range(n_params, n_params + len(out_names)))

    def _body(*args):
        operands = list(args)
        if partition_name is not None:
            operands.append(B.partition_id_tensor())
        outs = B._bass_exec_p.bind(
            *operands,
            out_avals=tuple(out_avals),
            in_names=tuple(all_in_names),
            out_names=tuple(out_names),
            lowering_input_output_aliases=(),
            sim_require_finite=True,
            sim_require_nnan=True,
            nc=nc,
        )
        return tuple(outs)

    devices = jax.devices()[:n_cores]
    mesh = Mesh(np.asarray(devices), ("core",))
    in_specs = (PartitionSpec("core"),) * (n_params + len(out_names))
    out_specs = (PartitionSpec("core"),) * len(out_names)
    fn = jax.jit(
        shard_map(
            _body, mesh=mesh, in_specs=in_specs, out_specs=out_specs,
            check_rep=False,
        ),
        donate_argnums=donate,
        keep_unused=True,
    )
    sh = jax.sharding.NamedSharding(mesh, PartitionSpec("core"))
    dev_in = [
        jax.device_put(
            np.concatenate([np.asarray(m[nm]) for m in in_maps], axis=0), sh
        )
        for nm in in_names
    ]

    def zeros():
        return [
            jax.device_put(
                jnp.zeros((n_cores * s[0],) + tuple(s[1:]), dt), sh
            )
            for (s, dt) in zero_shapes
        ]

    # warm up (compile + first exec)
    outs = fn(*dev_in, *zeros())
    jax.block_until_ready(outs)
    times = []
    for _ in range(iters):
        z = zeros()
        jax.block_until_ready(z)
        t0 = _t.perf_counter()
        outs = fn(*dev_in, *z)
        jax.block_until_ready(outs)
        times.append(_t.perf_counter() - t0)
    return times


def kernel(**inputs) -> np.ndarray:
    x = np.asarray(inputs["x"], dtype=np.float32)
    W = np.asarray(inputs["W"], dtype=np.float32)
    Es = [np.asarray(inputs[f"E{i}"], dtype=np.float32) for i in range(4)]
    bias = np.asarray(inputs["bias"], dtype=np.float32)
    ms = [np.asarray(inputs[f"m{i}"]).astype(bool) for i in range(4)]

    trace = bool(int(os.environ.get("HGRAPH_TRACE", "0")))
    out = _run(x, W, Es[1:], bias, ms[1:], N_CORES, trace=trace)

    f = W.shape[2]
    n = x.shape[1]
    # Safety net 1: hop 0 assumes m0 == I (structurally true for this module).
    if not np.array_equal(ms[0], np.eye(n, dtype=bool)):
        s0 = np.where(ms[0], Es[0], NEG)
        s0 = s0 - s0.max(axis=1, keepdims=True)
        p0 = np.exp(s0)
        a0 = p0 / p0.sum(axis=1, keepdims=True)
        h0 = np.einsum("bnf,fo->bno", x, W[0])
        out[:, :, 0:f] = np.einsum("ij,bjo->bio", a0, h0) + bias[None, None, :f]
    # Safety net 2: all-masked rows (softmax -> uniform; device would give NaN).
    for k in range(1, 4):
        empty = ~ms[k].any(axis=1)
        if empty.any():
            hk = np.einsum("bnf,fo->bno", x, W[k])
            unif = hk.mean(axis=1)  # [B, f]
            idx = np.where(empty)[0]
            out[:, idx, k * f:(k + 1) * f] = unif[:, None, :] + bias[None, None, k * f:(k + 1) * f]
    return out


def bench_pipelined(nc, in_maps, k=16):
    """Issue k executions back-to-back without host sync; returns total wall.
    If dispatches pipeline, slope vs k isolates device execution time."""
    import time as _t

    import jax
    import jax.numpy as jnp
    from jax.experimental.shard_map import shard_map
    from jax.sharding import Mesh, PartitionSpec

    import concourse.mybir as mb
    from concourse import bass2jax as B

    B.install_neuronx_cc_hook()
    n_cores = len(in_maps)
    partition_name = nc.partition_id_tensor.name if nc.partition_id_tensor else None
    in_names, out_names, out_avals, zero_shapes = [], [], [], []
    for alloc in nc.m.functions[0].allocations:
        if not isinstance(alloc, mb.MemoryLocationSet):
            continue
        name = alloc.memorylocations[0].name
        if alloc.kind == "ExternalInput":
            if name != partition_name:
                in_names.append(name)
        elif alloc.kind == "ExternalOutput":
            shape = tuple(alloc.tensor_shape)
            dtype = mb.dt.np(alloc.dtype)
            out_names.append(name)
            out_avals.append(jax.core.ShapedArray(shape, dtype))
            zero_shapes.append((shape, dtype))
    n_params = len(in_names)
    all_in_names = list(in_names) + list(out_names)
    if partition_name is not None:
        all_in_names.append(partition_name)
    donate = tuple(range(n_params, n_params + len(out_names)))

    def _body(*args):
        operands = list(args)
        if partition_name is not None:
            operands.append(B.partition_id_tensor())
        outs = B._bass_exec_p.bind(
            *operands,
            out_avals=tuple(out_avals),
            in_names=tuple(all_in_names),
            out_names=tuple(out_names),
            lowering_input_output_aliases=(),
            sim_require_finite=True,
            sim_require_nnan=True,
            nc=nc,
        )
        return tuple(outs)

    devices = jax.devices()[:n_cores]
    mesh = Mesh(np.asarray(devices), ("core",))
    in_specs = (PartitionSpec("core"),) * (n_params + len(out_names))
    out_specs = (PartitionSpec("core"),) * len(out_names)
    fn = jax.jit(
        shard_map(_body, mesh=mesh, in_specs=in_specs, out_specs=out_specs,
                  check_rep=False),
        donate_argnums=donate, keep_unused=True,
    )
    sh = jax.sharding.NamedSharding(mesh, PartitionSpec("core"))
    dev_in = [
        jax.device_put(
            np.concatenate([np.asarray(m[nm]) for m in in_maps], axis=0), sh
        )
        for nm in in_names
    ]

    def zeros():
        return [
            jax.device_put(jnp.zeros((n_cores * s[0],) + tuple(s[1:]), dt), sh)
            for (s, dt) in zero_shapes
        ]

    outs = fn(*dev_in, *zeros())
    jax.block_until_ready(outs)
    zs = [zeros() for _ in range(k)]
    for z in zs:
        jax.block_until_ready(z)
    t0 = _t.perf_counter()
    res = []
    for z in zs:
        res.append(fn(*dev_in, *z))
    jax.block_until_ready(res)
    return _t.perf_counter() - t0

